# revision 50
# baseline (speedup 1.0000x reference)
"""AdaptiveMixing Trainium2 kernel — 8-core data parallel, v2.

Per query n (M=256 per core):
  q  = LayerNorm(query[n]) * ln_w + ln_b
  h  = q @ w1.T + b1                      # [128]
  params = h @ w2.T + b2                  # [66560]
  cm = params[:65536].reshape(256, 256)
  sm = params[65536:].reshape(32, 32)
  o1 = gelu(x[n] @ cm + m_beta)           # [32, 256]
  o2 = gelu(sm @ o1 + s_beta[:, None])    # [32, 256]
  out[n] = o2.reshape(8192) @ proj_w.T + proj_b

Design notes (v2, ~4.7x over the v1 baseline):
- All weight transposes happen on the host: w2 arrives as [dq][k][j][c]
  bf16, x as [ch][c][m][p] bf16, proj_w as [dh][f][o][e] bf16, w1 as w1^T.
  This removes ~1500 on-device PE transposes + PSUM drains and halves the
  w2 HBM traffic (34 MB -> 17 MB per core).
- cm generation: 512 bf16 matmuls (w2 slab stationary, persistent hT
  moving, N=256), each drained PSUM->SBUF by a fused bias+cast
  tensor_scalar/activation, alternating DVE/Act across different banks
  (same-bank DVE+Act reads serialize on TRN2). 4 PSUM banks keep the
  drains back-to-back on both engines.
- Mixing-1 is flipped (cm as stationary, xs^T as moving) so d lands on
  partitions: m_beta folds into the gelu1 per-partition bias and two
  4-group batches share one [128, 512] PSUM bank via tile_position
  col-offset, halving gelu1 instruction count.
- Mixing-2 is merged with the transpose back to [d, (q, op)]: one matmul
  per 4-query group with the block-diagonal sm^T as the MOVING operand
  (stationary = gelu(o1)), s_beta injected by a K=1 ones-row matmul that
  opens the PSUM accumulation group.
- proj accumulates into one held PSUM bank over 64 f-chunks; the dh0 half
  interleaves into quarter-2's generation stream, the dh1 first m-half
  into quarter-3's mix tail.
- The mix phase is software-pipelined 2 batch-pairs deep so cm_sb reads
  retire early enough for the next quarter's generation to overlap.
"""

import sys

sys.path.insert(0, "/opt/trn_rl_repo")

import numpy as np

import concourse.bass as bass
import concourse.mybir as mybir
import concourse.tile as tile
from concourse.bass_utils import run_bass_kernel_spmd
from concourse.masks import make_identity

F32 = mybir.dt.float32
BF16 = mybir.dt.bfloat16
AF = mybir.ActivationFunctionType

B, N, P, C = 2, 1024, 32, 256
OP, HID = 32, 128
CC = C * C
TOTAL = CC + OP * P  # 66560
NCORES = 8
M = (B * N) // NCORES  # 256 queries per core
NG = M // 4            # 64 groups of 4 queries


def _ap(handle, offset, ap):
    return bass.AP(tensor=handle.ap().tensor, offset=offset, ap=[list(p) for p in ap])


def build(nc: bass.Bass):
    d_query = nc.dram_tensor("query", [M, C], F32, kind="ExternalInput")
    d_xh = nc.dram_tensor("xh", [2, 128, M * P], BF16, kind="ExternalInput")
    d_w2cm = nc.dram_tensor("w2cm", [4, 128, 64 * C], BF16, kind="ExternalInput")
    d_w2sm = nc.dram_tensor("w2sm", [128, 1024], BF16, kind="ExternalInput")
    d_w1t = nc.dram_tensor("w1t", [2, 128, 128], BF16, kind="ExternalInput")
    d_b1 = nc.dram_tensor("b1v", [HID], F32, kind="ExternalInput")
    d_b2cm = nc.dram_tensor("b2cmv", [128, 2 * C], F32, kind="ExternalInput")
    d_b2sm = nc.dram_tensor("b2smv", [128, 8], F32, kind="ExternalInput")
    d_lnw = nc.dram_tensor("ln_w", [C], F32, kind="ExternalInput")
    d_lnb = nc.dram_tensor("ln_b", [C], F32, kind="ExternalInput")
    d_mbq = nc.dram_tensor("mbq", [128, 4], F32, kind="ExternalInput")
    d_sbr = nc.dram_tensor("sbrow", [512], BF16, kind="ExternalInput")
    d_pwt = nc.dram_tensor("pwt", [2, 128, OP * C], BF16, kind="ExternalInput")
    d_pjb = nc.dram_tensor("pjb", [128, 2], F32, kind="ExternalInput")
    d_y = nc.dram_tensor("y", [128, 2 * M], BF16, kind="ExternalOutput")

    from contextlib import ExitStack
    with tile.TileContext(nc) as tc, ExitStack() as ctx:
        _body(ctx, nc, tc, d_query, d_xh, d_w2cm, d_w2sm, d_w1t, d_b1, d_b2cm,
              d_b2sm, d_lnw, d_lnb, d_mbq, d_sbr, d_pwt, d_pjb, d_y)
    return nc


def _body(ctx, nc, tc, d_query, d_xh, d_w2cm, d_w2sm, d_w1t, d_b1, d_b2cm,
          d_b2sm, d_lnw, d_lnb, d_mbq, d_sbr, d_pwt, d_pjb, d_y):
    singles = ctx.enter_context(tc.tile_pool(name="singles", bufs=1))
    tmp = ctx.enter_context(tc.tile_pool(name="tmp", bufs=4))
    w2st_p = ctx.enter_context(tc.tile_pool(name="w2st", bufs=8))
    o1g_p = ctx.enter_context(tc.tile_pool(name="o1g", bufs=8))
    m1g_p = ctx.enter_context(tc.tile_pool(name="m1g", bufs=3))
    ps_gen = ctx.enter_context(tc.tile_pool(name="ps_gen", bufs=4, space="PSUM"))
    ps_mix = ctx.enter_context(tc.tile_pool(name="ps_mix", bufs=3, space="PSUM"))
    ps_out = ctx.enter_context(tc.tile_pool(name="ps_out", bufs=1, space="PSUM"))

    # ---------------- constants / small DMAs ----------------
    ident_f = singles.tile([128, 128], F32)
    make_identity(nc, ident_f)
    ident_bf = singles.tile([128, 128], BF16)
    nc.vector.tensor_copy(out=ident_bf, in_=ident_f)

    # query first: the LayerNorm -> hT chain gates all cm generation
    qts, mvs, rstds = [], [], []
    for mt in range(2):
        qt = tmp.tile([128, C], F32, tag=f"qt{mt}")
        nc.sync.dma_start(out=qt, in_=_ap(d_query, mt * 128 * C, [[C, 128], [1, C]]))
        qts.append(qt)

    lnw_b = singles.tile([128, C], F32)
    nc.sync.dma_start(out=lnw_b, in_=_ap(d_lnw, 0, [[0, 128], [1, C]]))
    lnb_b = singles.tile([128, C], F32)
    nc.sync.dma_start(out=lnb_b, in_=_ap(d_lnb, 0, [[0, 128], [1, C]]))
    b1_sb = singles.tile([128, 1], F32)
    nc.sync.dma_start(out=b1_sb, in_=_ap(d_b1, 0, [[1, 128], [0, 1]]))
    b2cm_sb = singles.tile([128, 2, C], F32)
    nc.sync.dma_start(out=b2cm_sb, in_=_ap(d_b2cm, 0, [[2 * C, 128], [1, 2 * C]]))
    ones_sb = singles.tile([1, 64], BF16)
    nc.vector.memset(ones_sb, 1.0)
    eps_sb = singles.tile([128, 1], F32)
    nc.vector.memset(eps_sb, 1e-6)
    w1t_sb = singles.tile([128, 2, 128], BF16)
    nc.sync.dma_start(out=w1t_sb, in_=_ap(
        d_w1t, 0, [[128, 128], [128 * 128, 2], [1, 128]]))

    # ---------------- big persistent buffers ----------------
    cm_sb = singles.tile([128, 2, 64, M], BF16)     # [c_low, ch, j, m]
    xh_sb = singles.tile([128, 2, M, P], BF16)      # [c_low, ch, m, p]
    S_sb = singles.tile([128, NG, 128], BF16)       # [(q,p), g, (r,o)]
    flat2 = singles.tile([128, OP, M], BF16)        # [(parity,dl), o, m]
    pw_sb = singles.tile([128, OP, C], BF16)        # [f, o, e] one d-half
    qn_bf = singles.tile([128, 2, C], BF16)
    qnT = singles.tile([128, 2, M], BF16)
    hT = singles.tile([128, M], BF16)
    outE = singles.tile([128, 2, M], BF16)

    def stage_w2(dq):
        tiles = []
        for h in range(8):
            st = w2st_p.tile([128, 8, C], BF16, tag="w2st")
            nc.sync.dma_start(out=st, in_=_ap(
                d_w2cm, dq * 128 * 64 * C + h * 8 * C,
                [[64 * C, 128], [1, 8 * C]]))
            tiles.append(st)
        return tiles

    # quarter-0 w2 must beat w2sm and the big xh transfer into the queue
    w2st_cur = stage_w2(0)
    w2sm_sb = singles.tile([128, 1024], BF16)
    nc.sync.dma_start(out=w2sm_sb, in_=_ap(d_w2sm, 0, [[1024, 128], [1, 1024]]))
    b2sm_sb = singles.tile([128, 8], F32)
    nc.sync.dma_start(out=b2sm_sb, in_=_ap(d_b2sm, 0, [[8, 128], [1, 8]]))
    mbq_sb = singles.tile([128, 4], F32)
    nc.sync.dma_start(out=mbq_sb, in_=_ap(d_mbq, 0, [[4, 128], [1, 4]]))
    sbr_sb = singles.tile([1, 512], BF16)
    nc.sync.dma_start(out=sbr_sb, in_=_ap(d_sbr, 0, [[0, 1], [1, 512]]))
    pjb_sb = singles.tile([128, 2], F32)
    nc.sync.dma_start(out=pjb_sb, in_=_ap(d_pjb, 0, [[2, 128], [1, 2]]))
    for ch in range(2):
        nc.sync.dma_start(out=xh_sb[:, ch, :, :], in_=_ap(
            d_xh, ch * 128 * M * P, [[M * P, 128], [1, M * P]]))

    nc.gpsimd.memset(S_sb, 0.0)

    # ---------------- LayerNorm -> qn (bf16) ----------------
    for mt in range(2):
        stats = tmp.tile([128, 6], F32, tag="st")
        nc.vector.bn_stats(out=stats, in_=qts[mt])
        mv = tmp.tile([128, 2], F32, tag=f"mv{mt}")
        nc.vector.bn_aggr(out=mv, in_=stats)
        mvs.append(mv)
    for mt in range(2):
        rstd = tmp.tile([128, 1], F32, tag=f"rs{mt}")
        nc.scalar.activation(out=rstd, in_=mvs[mt][:, 1:2], func=AF.Sqrt,
                             bias=eps_sb, scale=1.0)
        nc.vector.reciprocal(out=rstd, in_=rstd)
        rstds.append(rstd)
    for mt in range(2):
        qt = qts[mt]
        nc.vector.tensor_scalar(out=qt, in0=qt, scalar1=mvs[mt][:, 0:1],
                                scalar2=rstds[mt],
                                op0=mybir.AluOpType.subtract,
                                op1=mybir.AluOpType.mult)
        nc.vector.tensor_mul(out=qt, in0=qt, in1=lnw_b)
        nc.vector.tensor_add(out=qn_bf[:, mt, :], in0=qt, in1=lnb_b)

    # qnT [c_low, ch, m]
    for mt in range(2):
        for ch in range(2):
            pt = ps_mix.tile([128, 2, 64], BF16, tag="mixps")
            ptv = pt.rearrange("p a b -> p (a b)")
            nc.tensor.transpose(ptv, qn_bf[:, mt, 128 * ch:128 * (ch + 1)], ident_bf)
            nc.vector.tensor_copy(out=qnT[:, ch, 128 * mt:128 * (mt + 1)], in_=ptv)

    # hT [k, m] = w1 @ qn.T + b1
    ps_h = ps_gen.tile([128, M], F32, tag="gen")
    for ch in range(2):
        nc.tensor.matmul(ps_h, w1t_sb[:, ch, :], qnT[:, ch, :],
                         start=(ch == 0), stop=(ch == 1))
    nc.vector.tensor_scalar_add(out=hT, in0=ps_h, scalar1=b1_sb)

    # ---------------- sm params -> S (block-diag smT + b2) ----------------
    def emit_smgen():
        for c4 in range(8):
            pg = ps_gen.tile([128, M], F32, tag="gen")
            nc.tensor.matmul(pg, w2sm_sb[:, 128 * c4:128 * (c4 + 1)], hT,
                             start=True, stop=True)
            for oi in range(4):
                op = 4 * c4 + oi
                for r in range(4):
                    src = pg[32 * oi:32 * oi + 32, :].rearrange(
                        "p (g r) -> p g r", r=4)
                    dst = S_sb[32 * r:32 * r + 32, :, 32 * r + op]
                    bias = b2sm_sb[32 * oi:32 * oi + 32, c4:c4 + 1]
                    if (oi + r) % 2 == 0:
                        nc.vector.tensor_scalar_add(out=dst, in0=src[:, :, r],
                                                    scalar1=bias)
                    else:
                        nc.scalar.activation(out=dst, in_=src[:, :, r],
                                             func=AF.Identity, bias=bias,
                                             scale=1.0)

    # ---------------- main loop over d-quarters ----------------
    acc = ps_out.tile([128, 2, M], F32)  # [e_low, et, m], held across loop

    def emit_mix1_pair(dq, pi):
        """64 matmuls for batch pair (2pi, 2pi+1) into one [128, 512] bank
        (odd batch lands on rows 64-127 via tile_position col-offset), then a
        single full-width gelu1 -> o1g [128, 4, 128]."""
        pm1 = ps_mix.tile([128, 512], F32, tag="mixps")
        for half in range(2):
            bi = 2 * pi + half
            r0 = 64 * half
            tp_pos = (0, 64) if half else (0, 0)
            for gi in range(4):
                g = 4 * bi + gi
                for jq in range(4):
                    m = 4 * g + jq
                    col = 128 * gi + 32 * jq
                    for ch in range(2):
                        first = (gi == 0 and jq == 0 and ch == 0)
                        last = (gi == 3 and jq == 3 and ch == 1)
                        nc.tensor.matmul(
                            pm1[r0:r0 + 64, col:col + 32],
                            cm_sb[:, ch, :, m], xh_sb[:, ch, m, :],
                            start=first, stop=last, tile_position=tp_pos)
        o1g = o1g_p.tile([128, 4, 128], BF16, tag="o1g")
        nc.scalar.activation(out=o1g, in_=pm1, func=AF.Gelu,
                             bias=mbq_sb[:, dq:dq + 1], scale=1.0)
        return o1g

    def emit_mix2(dq, bi, o1g, half):
        """Transposes, mix2 (merged with transpose) and gelu2 -> flat2."""
        parity = dq % 2
        r0 = 64 * half
        idn = ident_bf[r0:r0 + 64, r0:r0 + 64]
        m1g = m1g_p.tile([128, 4, 64], BF16, tag="m1g")
        pt = ps_mix.tile([128, 4, 64], BF16, tag="mixps")
        for i in range(4):
            nc.tensor.transpose(pt[:, i, :], o1g[r0:r0 + 64, i, :], idn)
        nc.vector.tensor_copy(out=m1g, in_=pt)
        o2p = ps_gen.tile([64, 512], F32, tag="gen")
        nc.tensor.matmul(o2p, ones_sb, sbr_sb, start=True, stop=False)
        for gi in range(4):
            g = 4 * bi + gi
            nc.tensor.matmul(o2p[:, 128 * gi:128 * (gi + 1)],
                             m1g[:, gi, :], S_sb[:, g, :],
                             start=False, stop=(gi == 3))
        dst = flat2[64 * parity:64 * parity + 64, :, 16 * bi:16 * bi + 16]
        dst = dst.rearrange("d o m -> d m o")
        nc.scalar.activation(out=dst, in_=o2p, func=AF.Gelu, scale=1.0)

    def emit_proj(dh, o):
        for et in range(2):
            nc.tensor.matmul(
                acc[:, et, :], pw_sb[:, o, 128 * et:128 * (et + 1)],
                flat2[:, o, :],
                start=(dh == 0 and o == 0 and et == 0),
                stop=False)

    def emit_proj_half(o, mh):
        for et in range(2):
            nc.tensor.matmul(
                acc[:, et, 128 * mh:128 * (mh + 1)],
                pw_sb[:, o, 128 * et:128 * (et + 1)],
                flat2[:, o, 128 * mh:128 * (mh + 1)],
                start=False,
                stop=(o == OP - 1 and et == 1 and mh == 1))

    for dq in range(4):
        parity = dq % 2
        w2st = w2st_cur
        # prefetch proj weights for this d-half at start of odd quarters
        if parity == 1:
            dh = dq // 2
            nc.sync.dma_start(out=pw_sb, in_=_ap(
                d_pwt, dh * 128 * OP * C, [[OP * C, 128], [1, OP * C]]))

        # cm generation: 128 MMs + 128 biased drains. One MM per PSUM bank;
        # consecutive drains alternate DVE/Act across different banks (same-
        # bank DVE+Act reads would serialize). The previous d-half's proj
        # matmuls interleave into the gen stream to fill PE gaps.
        for jl in range(64):
            st = w2st[jl // 8]
            if dq == 2 and jl % 2 == 0:
                emit_proj(0, jl // 2)
            d0 = dq * 64 + jl
            for ch in range(2):
                pc = ps_gen.tile([128, M], F32, tag="gen")
                nc.tensor.matmul(pc, st[:, jl % 8, 128 * ch:128 * (ch + 1)],
                                 hT, start=True, stop=True)
                bias = b2cm_sb[:, ch, d0:d0 + 1]
                if (2 * jl + ch) % 2 == 0:
                    nc.vector.tensor_scalar_add(out=cm_sb[:, ch, jl, :], in0=pc,
                                                scalar1=bias)
                else:
                    nc.scalar.activation(out=cm_sb[:, ch, jl, :], in_=pc,
                                         func=AF.Identity, bias=bias,
                                         scale=1.0)

        # stage the next quarter's w2 now; transfers overlap the mix phase
        if dq < 3:
            w2st_cur = stage_w2(dq + 1)

        # sm params for the S matrix are produced once, tucked after the
        # first quarter's gen so their drains don't block startup
        if dq == 0:
            emit_smgen()

        # mixing, 8 batch-pairs of 8 groups, software-pipelined 2 pairs deep:
        # cm_sb reads (mix1 ldweights) finish early in the mix phase so the
        # next quarter's generation + drains overlap the mix2 tail.
        SKEW = 4

        def after_mix2(p):
            # on dq3, once pairs 0..2+p are drained (m 0..127 covered by
            # pairs 0-3), interleave the first m-half of the final proj
            if dq == 3 and 3 <= p <= 6:
                for o in range(8 * (p - 3), 8 * (p - 2)):
                    emit_proj_half(o, 0)

        o1gs = {}
        for pi in range(8):
            o1gs[pi] = emit_mix1_pair(dq, pi)
            if pi >= SKEW:
                og = o1gs.pop(pi - SKEW)
                for half in range(2):
                    emit_mix2(dq, 2 * (pi - SKEW) + half, og, half)
                after_mix2(pi - SKEW)
        for pi in range(8 - SKEW, 8):
            og = o1gs.pop(pi)
            for half in range(2):
                emit_mix2(dq, 2 * pi + half, og, half)
            after_mix2(pi)

    # final d-half proj (dh=1): second m-half
    for o in range(OP):
        emit_proj_half(o, 1)

    # ---------------- epilogue ----------------
    # y is stored e-major ([e_low, et, m], bf16); the host transposes back
    for et in range(2):
        nc.scalar.activation(out=outE[:, et, :], in_=acc[:, et, :],
                             func=AF.Identity, bias=pjb_sb[:, et:et + 1],
                             scale=1.0)
    nc.sync.dma_start(out=_ap(d_y, 0, [[2 * M, 128], [1, 2 * M]]), in_=outE)


def legalize_sync_waits(nc, max_waits=1):
    """This walrus build accepts only one sync wait per instruction; move
    extras onto preceding same-engine NoOps."""
    ctr = 0
    for f in nc.m.functions:
        for bb in f.blocks:
            out, changed = [], False
            for inst in bb.instructions:
                si = inst.sync_info
                if si is not None and si.on_wait and len(si.on_wait) > max_waits:
                    waits = list(si.on_wait)
                    for w in waits[:-max_waits]:
                        ctr += 1
                        n = mybir.InstNoOp(name=f"lw_nop_{ctr}", ins=[], outs=[])
                        n.engine = inst.engine
                        n.sync_info = mybir.SyncInfo(on_update=[], on_wait=[w])
                        out.append(n)
                    inst.sync_info = mybir.SyncInfo(
                        on_update=list(si.on_update or []),
                        on_wait=waits[-max_waits:])
                    changed = True
                out.append(inst)
            if changed:
                bb.instructions = out
    return ctr


_CACHE = {}


def _get_nc():
    if "nc" not in _CACHE:
        nc = bass.Bass()
        build(nc)
        legalize_sync_waits(nc)
        _CACHE["nc"] = nc
    return _CACHE["nc"]


def _prep_shared(inputs):
    import ml_dtypes
    bf16 = ml_dtypes.bfloat16
    f32 = np.float32
    w2 = np.asarray(inputs["w2"], f32)
    w1 = np.asarray(inputs["w1"], f32)
    b2 = np.asarray(inputs["b2"], f32)
    m_beta = np.asarray(inputs["m_beta"], f32)
    s_beta = np.asarray(inputs["s_beta"], f32)
    proj_w = np.asarray(inputs["proj_w"], f32)
    proj_b = np.asarray(inputs["proj_b"], f32)
    sh = {}
    # w2cm[dq, k, j, c] = w2[c*256 + dq*64 + j, k]
    sh["w2cm"] = np.ascontiguousarray(
        w2[:CC].reshape(C, 4, 64, HID).transpose(1, 3, 2, 0)
    ).astype(bf16).reshape(4, 128, 64 * C)
    sh["w2sm"] = np.ascontiguousarray(w2[CC:].T).astype(bf16)
    sh["w1t"] = np.ascontiguousarray(
        w1.reshape(HID, 2, 128).transpose(1, 2, 0)).astype(bf16)
    sh["b1v"] = np.asarray(inputs["b1"], f32)
    sh["b2cmv"] = np.ascontiguousarray(
        b2[:CC].reshape(2, 128, C).transpose(1, 0, 2)).reshape(128, 2 * C)
    sh["b2smv"] = np.ascontiguousarray(b2[CC:].reshape(8, 128).T)
    sh["ln_w"] = np.asarray(inputs["ln_w"], f32)
    sh["ln_b"] = np.asarray(inputs["ln_b"], f32)
    # mbq[dl + 64*h, dq] = m_beta[dq*64 + dl]
    mb = m_beta.reshape(4, 64).T  # [64, 4]
    sh["mbq"] = np.ascontiguousarray(np.concatenate([mb, mb], axis=0))
    sh["sbrow"] = np.tile(s_beta, 16).astype(bf16)
    # pwt[dh, f, o, e] = proj_w[e, o*256 + dh*128 + f]
    sh["pwt"] = np.ascontiguousarray(
        proj_w.reshape(C, OP, 2, 128).transpose(2, 3, 1, 0)
    ).astype(bf16).reshape(2, 128, OP * C)
    sh["pjb"] = np.ascontiguousarray(proj_b.reshape(2, 128).T)
    return sh


def kernel(**inputs):
    import ml_dtypes
    bf16 = ml_dtypes.bfloat16
    nc = _get_nc()
    x = np.asarray(inputs["x"], np.float32).reshape(B * N, P, C)
    query = np.asarray(inputs["query"], np.float32).reshape(B * N, C)
    # xh[ch, cl, m, p] = x[m, p, ch*128 + cl]
    xh = np.ascontiguousarray(
        x.reshape(B * N, P, 2, 128).transpose(2, 3, 0, 1)).astype(bf16)
    shared = _prep_shared(inputs)
    in_maps = []
    for c in range(NCORES):
        mmap = dict(shared)
        mmap["xh"] = np.ascontiguousarray(
            xh[:, :, c * M:(c + 1) * M, :]).reshape(2, 128, M * P)
        mmap["query"] = np.ascontiguousarray(query[c * M:(c + 1) * M])
        in_maps.append(mmap)
    res = run_bass_kernel_spmd(nc, in_maps, core_ids=list(range(NCORES)))
    outs = []
    for c in range(NCORES):
        ye = np.asarray(res.results[c]["y"]).reshape(128, 2, M)
        outs.append(ye.transpose(2, 1, 0).reshape(M, C).astype(np.float32))
    return np.concatenate(outs, axis=0).reshape(B, N, C)


if __name__ == "__main__":
    rng = np.random.default_rng(0)
    ins = {
        "x": rng.standard_normal((B, N, 1, P, C), dtype=np.float32),
        "query": rng.standard_normal((B, N, C), dtype=np.float32),
        "ln_w": np.full((C,), C ** -0.5, np.float32),
        "ln_b": np.zeros((C,), np.float32),
        "w1": (rng.standard_normal((HID, C)) * 0.02).astype(np.float32),
        "b1": np.zeros((HID,), np.float32),
        "w2": (rng.standard_normal((TOTAL, HID)) * 0.02).astype(np.float32),
        "b2": (rng.standard_normal((TOTAL,)) * 0.05).astype(np.float32),
        "m_beta": np.zeros((C,), np.float32),
        "s_beta": np.zeros((OP,), np.float32),
        "proj_w": (rng.standard_normal((C, OP * C)) * 0.02).astype(np.float32),
        "proj_b": np.zeros((C,), np.float32),
    }
    out = kernel(**ins)
    print("ran", out.shape, out.dtype)


# revision 51
# speedup vs baseline: 1.0020x; 1.0020x over previous
"""AdaptiveMixing Trainium2 kernel — 8-core data parallel, v2.

Per query n (M=256 per core):
  q  = LayerNorm(query[n]) * ln_w + ln_b
  h  = q @ w1.T + b1                      # [128]
  params = h @ w2.T + b2                  # [66560]
  cm = params[:65536].reshape(256, 256)
  sm = params[65536:].reshape(32, 32)
  o1 = gelu(x[n] @ cm + m_beta)           # [32, 256]
  o2 = gelu(sm @ o1 + s_beta[:, None])    # [32, 256]
  out[n] = o2.reshape(8192) @ proj_w.T + proj_b

Design notes (v2, ~4.7x over the v1 baseline):
- All weight transposes happen on the host: w2 arrives as [dq][k][j][c]
  bf16, x as [ch][c][m][p] bf16, proj_w as [dh][f][o][e] bf16, w1 as w1^T.
  This removes ~1500 on-device PE transposes + PSUM drains and halves the
  w2 HBM traffic (34 MB -> 17 MB per core).
- cm generation: 512 bf16 matmuls (w2 slab stationary, persistent hT
  moving, N=256), each drained PSUM->SBUF by a fused bias+cast
  tensor_scalar/activation, alternating DVE/Act across different banks
  (same-bank DVE+Act reads serialize on TRN2). 4 PSUM banks keep the
  drains back-to-back on both engines.
- Mixing-1 is flipped (cm as stationary, xs^T as moving) so d lands on
  partitions: m_beta folds into the gelu1 per-partition bias and two
  4-group batches share one [128, 512] PSUM bank via tile_position
  col-offset, halving gelu1 instruction count.
- Mixing-2 is merged with the transpose back to [d, (q, op)]: one matmul
  per 4-query group with the block-diagonal sm^T as the MOVING operand
  (stationary = gelu(o1)), s_beta injected by a K=1 ones-row matmul that
  opens the PSUM accumulation group.
- proj accumulates into one held PSUM bank over 64 f-chunks; the dh0 half
  interleaves into quarter-2's generation stream, the dh1 first m-half
  into quarter-3's mix tail.
- The mix phase is software-pipelined 2 batch-pairs deep so cm_sb reads
  retire early enough for the next quarter's generation to overlap.
"""

import sys

sys.path.insert(0, "/opt/trn_rl_repo")

import numpy as np

import concourse.bass as bass
import concourse.mybir as mybir
import concourse.tile as tile
from concourse.bass_utils import run_bass_kernel_spmd
from concourse.masks import make_identity

F32 = mybir.dt.float32
BF16 = mybir.dt.bfloat16
AF = mybir.ActivationFunctionType

B, N, P, C = 2, 1024, 32, 256
OP, HID = 32, 128
CC = C * C
TOTAL = CC + OP * P  # 66560
NCORES = 8
M = (B * N) // NCORES  # 256 queries per core
NG = M // 4            # 64 groups of 4 queries


def _ap(handle, offset, ap):
    return bass.AP(tensor=handle.ap().tensor, offset=offset, ap=[list(p) for p in ap])


def build(nc: bass.Bass):
    d_query = nc.dram_tensor("query", [M, C], F32, kind="ExternalInput")
    d_xh = nc.dram_tensor("xh", [2, 128, M * P], BF16, kind="ExternalInput")
    d_w2cm = nc.dram_tensor("w2cm", [4, 128, 64 * C], BF16, kind="ExternalInput")
    d_w2sm = nc.dram_tensor("w2sm", [128, 1024], BF16, kind="ExternalInput")
    d_w1t = nc.dram_tensor("w1t", [2, 128, 128], BF16, kind="ExternalInput")
    d_b1 = nc.dram_tensor("b1v", [HID], F32, kind="ExternalInput")
    d_b2cm = nc.dram_tensor("b2cmv", [128, 2 * C], F32, kind="ExternalInput")
    d_b2sm = nc.dram_tensor("b2smv", [128, 8], F32, kind="ExternalInput")
    d_lnw = nc.dram_tensor("ln_w", [C], F32, kind="ExternalInput")
    d_lnb = nc.dram_tensor("ln_b", [C], F32, kind="ExternalInput")
    d_mbq = nc.dram_tensor("mbq", [128, 4], F32, kind="ExternalInput")
    d_sbr = nc.dram_tensor("sbrow", [512], BF16, kind="ExternalInput")
    d_pwt = nc.dram_tensor("pwt", [2, 128, OP * C], BF16, kind="ExternalInput")
    d_pjb = nc.dram_tensor("pjb", [128, 2], F32, kind="ExternalInput")
    d_y = nc.dram_tensor("y", [128, 2 * M], BF16, kind="ExternalOutput")

    from contextlib import ExitStack
    with tile.TileContext(nc) as tc, ExitStack() as ctx:
        _body(ctx, nc, tc, d_query, d_xh, d_w2cm, d_w2sm, d_w1t, d_b1, d_b2cm,
              d_b2sm, d_lnw, d_lnb, d_mbq, d_sbr, d_pwt, d_pjb, d_y)
    return nc


def _body(ctx, nc, tc, d_query, d_xh, d_w2cm, d_w2sm, d_w1t, d_b1, d_b2cm,
          d_b2sm, d_lnw, d_lnb, d_mbq, d_sbr, d_pwt, d_pjb, d_y):
    singles = ctx.enter_context(tc.tile_pool(name="singles", bufs=1))
    tmp = ctx.enter_context(tc.tile_pool(name="tmp", bufs=4))
    w2st_p = ctx.enter_context(tc.tile_pool(name="w2st", bufs=8))
    o1g_p = ctx.enter_context(tc.tile_pool(name="o1g", bufs=8))
    m1g_p = ctx.enter_context(tc.tile_pool(name="m1g", bufs=4))
    ps_gen = ctx.enter_context(tc.tile_pool(name="ps_gen", bufs=4, space="PSUM"))
    ps_mix = ctx.enter_context(tc.tile_pool(name="ps_mix", bufs=3, space="PSUM"))
    ps_out = ctx.enter_context(tc.tile_pool(name="ps_out", bufs=1, space="PSUM"))

    # ---------------- constants / small DMAs ----------------
    ident_f = singles.tile([128, 128], F32)
    make_identity(nc, ident_f)
    ident_bf = singles.tile([128, 128], BF16)
    nc.vector.tensor_copy(out=ident_bf, in_=ident_f)

    # query first: the LayerNorm -> hT chain gates all cm generation
    qts, mvs, rstds = [], [], []
    for mt in range(2):
        qt = tmp.tile([128, C], F32, tag=f"qt{mt}")
        nc.sync.dma_start(out=qt, in_=_ap(d_query, mt * 128 * C, [[C, 128], [1, C]]))
        qts.append(qt)

    lnw_b = singles.tile([128, C], F32)
    nc.sync.dma_start(out=lnw_b, in_=_ap(d_lnw, 0, [[0, 128], [1, C]]))
    lnb_b = singles.tile([128, C], F32)
    nc.sync.dma_start(out=lnb_b, in_=_ap(d_lnb, 0, [[0, 128], [1, C]]))
    b1_sb = singles.tile([128, 1], F32)
    nc.sync.dma_start(out=b1_sb, in_=_ap(d_b1, 0, [[1, 128], [0, 1]]))
    b2cm_sb = singles.tile([128, 2, C], F32)
    nc.sync.dma_start(out=b2cm_sb, in_=_ap(d_b2cm, 0, [[2 * C, 128], [1, 2 * C]]))
    ones_sb = singles.tile([1, 64], BF16)
    nc.vector.memset(ones_sb, 1.0)
    eps_sb = singles.tile([128, 1], F32)
    nc.vector.memset(eps_sb, 1e-6)
    w1t_sb = singles.tile([128, 2, 128], BF16)
    nc.sync.dma_start(out=w1t_sb, in_=_ap(
        d_w1t, 0, [[128, 128], [128 * 128, 2], [1, 128]]))

    # ---------------- big persistent buffers ----------------
    cm_sb = singles.tile([128, 2, 64, M], BF16)     # [c_low, ch, j, m]
    xh_sb = singles.tile([128, 2, M, P], BF16)      # [c_low, ch, m, p]
    S_sb = singles.tile([128, NG, 128], BF16)       # [(q,p), g, (r,o)]
    flat2 = singles.tile([128, OP, M], BF16)        # [(parity,dl), o, m]
    pw_sb = singles.tile([128, OP, C], BF16)        # [f, o, e] one d-half
    qn_bf = singles.tile([128, 2, C], BF16)
    qnT = singles.tile([128, 2, M], BF16)
    hT = singles.tile([128, M], BF16)
    outE = singles.tile([128, 2, M], BF16)

    def stage_w2(dq):
        tiles = []
        for h in range(8):
            st = w2st_p.tile([128, 8, C], BF16, tag="w2st")
            nc.sync.dma_start(out=st, in_=_ap(
                d_w2cm, dq * 128 * 64 * C + h * 8 * C,
                [[64 * C, 128], [1, 8 * C]]))
            tiles.append(st)
        return tiles

    # quarter-0 w2 must beat w2sm and the big xh transfer into the queue
    w2st_cur = stage_w2(0)
    w2sm_sb = singles.tile([128, 1024], BF16)
    nc.sync.dma_start(out=w2sm_sb, in_=_ap(d_w2sm, 0, [[1024, 128], [1, 1024]]))
    b2sm_sb = singles.tile([128, 8], F32)
    nc.sync.dma_start(out=b2sm_sb, in_=_ap(d_b2sm, 0, [[8, 128], [1, 8]]))
    mbq_sb = singles.tile([128, 4], F32)
    nc.sync.dma_start(out=mbq_sb, in_=_ap(d_mbq, 0, [[4, 128], [1, 4]]))
    sbr_sb = singles.tile([1, 512], BF16)
    nc.sync.dma_start(out=sbr_sb, in_=_ap(d_sbr, 0, [[0, 1], [1, 512]]))
    pjb_sb = singles.tile([128, 2], F32)
    nc.sync.dma_start(out=pjb_sb, in_=_ap(d_pjb, 0, [[2, 128], [1, 2]]))
    for ch in range(2):
        nc.sync.dma_start(out=xh_sb[:, ch, :, :], in_=_ap(
            d_xh, ch * 128 * M * P, [[M * P, 128], [1, M * P]]))

    nc.gpsimd.memset(S_sb, 0.0)

    # ---------------- LayerNorm -> qn (bf16) ----------------
    for mt in range(2):
        stats = tmp.tile([128, 6], F32, tag="st")
        nc.vector.bn_stats(out=stats, in_=qts[mt])
        mv = tmp.tile([128, 2], F32, tag=f"mv{mt}")
        nc.vector.bn_aggr(out=mv, in_=stats)
        mvs.append(mv)
    for mt in range(2):
        rstd = tmp.tile([128, 1], F32, tag=f"rs{mt}")
        nc.scalar.activation(out=rstd, in_=mvs[mt][:, 1:2], func=AF.Sqrt,
                             bias=eps_sb, scale=1.0)
        nc.vector.reciprocal(out=rstd, in_=rstd)
        rstds.append(rstd)
    for mt in range(2):
        qt = qts[mt]
        nc.vector.tensor_scalar(out=qt, in0=qt, scalar1=mvs[mt][:, 0:1],
                                scalar2=rstds[mt],
                                op0=mybir.AluOpType.subtract,
                                op1=mybir.AluOpType.mult)
        nc.vector.tensor_mul(out=qt, in0=qt, in1=lnw_b)
        nc.vector.tensor_add(out=qn_bf[:, mt, :], in0=qt, in1=lnb_b)

    # qnT [c_low, ch, m]
    for mt in range(2):
        for ch in range(2):
            pt = ps_mix.tile([128, 2, 64], BF16, tag="mixps")
            ptv = pt.rearrange("p a b -> p (a b)")
            nc.tensor.transpose(ptv, qn_bf[:, mt, 128 * ch:128 * (ch + 1)], ident_bf)
            nc.vector.tensor_copy(out=qnT[:, ch, 128 * mt:128 * (mt + 1)], in_=ptv)

    # hT [k, m] = w1 @ qn.T + b1
    ps_h = ps_gen.tile([128, M], F32, tag="gen")
    for ch in range(2):
        nc.tensor.matmul(ps_h, w1t_sb[:, ch, :], qnT[:, ch, :],
                         start=(ch == 0), stop=(ch == 1))
    nc.vector.tensor_scalar_add(out=hT, in0=ps_h, scalar1=b1_sb)

    # ---------------- sm params -> S (block-diag smT + b2) ----------------
    def emit_smgen():
        for c4 in range(8):
            pg = ps_gen.tile([128, M], F32, tag="gen")
            nc.tensor.matmul(pg, w2sm_sb[:, 128 * c4:128 * (c4 + 1)], hT,
                             start=True, stop=True)
            for oi in range(4):
                op = 4 * c4 + oi
                for r in range(4):
                    src = pg[32 * oi:32 * oi + 32, :].rearrange(
                        "p (g r) -> p g r", r=4)
                    dst = S_sb[32 * r:32 * r + 32, :, 32 * r + op]
                    bias = b2sm_sb[32 * oi:32 * oi + 32, c4:c4 + 1]
                    if (oi + r) % 2 == 0:
                        nc.vector.tensor_scalar_add(out=dst, in0=src[:, :, r],
                                                    scalar1=bias)
                    else:
                        nc.scalar.activation(out=dst, in_=src[:, :, r],
                                             func=AF.Identity, bias=bias,
                                             scale=1.0)

    # ---------------- main loop over d-quarters ----------------
    acc = ps_out.tile([128, 2, M], F32)  # [e_low, et, m], held across loop

    def emit_mix1_pair(dq, pi):
        """64 matmuls for batch pair (2pi, 2pi+1) into one [128, 512] bank
        (odd batch lands on rows 64-127 via tile_position col-offset), then a
        single full-width gelu1 -> o1g [128, 4, 128]."""
        pm1 = ps_mix.tile([128, 512], F32, tag="mixps")
        for half in range(2):
            bi = 2 * pi + half
            r0 = 64 * half
            tp_pos = (0, 64) if half else (0, 0)
            for gi in range(4):
                g = 4 * bi + gi
                for jq in range(4):
                    m = 4 * g + jq
                    col = 128 * gi + 32 * jq
                    for ch in range(2):
                        first = (gi == 0 and jq == 0 and ch == 0)
                        last = (gi == 3 and jq == 3 and ch == 1)
                        nc.tensor.matmul(
                            pm1[r0:r0 + 64, col:col + 32],
                            cm_sb[:, ch, :, m], xh_sb[:, ch, m, :],
                            start=first, stop=last, tile_position=tp_pos)
        o1g = o1g_p.tile([128, 4, 128], BF16, tag="o1g")
        nc.scalar.activation(out=o1g, in_=pm1, func=AF.Gelu,
                             bias=mbq_sb[:, dq:dq + 1], scale=1.0)
        return o1g

    def emit_mix2(dq, bi, o1g, half):
        """Transposes, mix2 (merged with transpose) and gelu2 -> flat2."""
        parity = dq % 2
        r0 = 64 * half
        idn = ident_bf[r0:r0 + 64, r0:r0 + 64]
        m1g = m1g_p.tile([128, 4, 64], BF16, tag="m1g")
        pt = ps_mix.tile([128, 4, 64], BF16, tag="mixps")
        for i in range(4):
            nc.tensor.transpose(pt[:, i, :], o1g[r0:r0 + 64, i, :], idn)
        nc.vector.tensor_copy(out=m1g, in_=pt)
        o2p = ps_gen.tile([64, 512], F32, tag="gen")
        nc.tensor.matmul(o2p, ones_sb, sbr_sb, start=True, stop=False)
        for gi in range(4):
            g = 4 * bi + gi
            nc.tensor.matmul(o2p[:, 128 * gi:128 * (gi + 1)],
                             m1g[:, gi, :], S_sb[:, g, :],
                             start=False, stop=(gi == 3))
        dst = flat2[64 * parity:64 * parity + 64, :, 16 * bi:16 * bi + 16]
        dst = dst.rearrange("d o m -> d m o")
        nc.scalar.activation(out=dst, in_=o2p, func=AF.Gelu, scale=1.0)

    def emit_proj(dh, o):
        for et in range(2):
            nc.tensor.matmul(
                acc[:, et, :], pw_sb[:, o, 128 * et:128 * (et + 1)],
                flat2[:, o, :],
                start=(dh == 0 and o == 0 and et == 0),
                stop=False)

    def emit_proj_half(o, mh):
        for et in range(2):
            nc.tensor.matmul(
                acc[:, et, 128 * mh:128 * (mh + 1)],
                pw_sb[:, o, 128 * et:128 * (et + 1)],
                flat2[:, o, 128 * mh:128 * (mh + 1)],
                start=False,
                stop=(o == OP - 1 and et == 1 and mh == 1))

    for dq in range(4):
        parity = dq % 2
        w2st = w2st_cur
        # prefetch proj weights for this d-half at start of odd quarters
        if parity == 1:
            dh = dq // 2
            nc.sync.dma_start(out=pw_sb, in_=_ap(
                d_pwt, dh * 128 * OP * C, [[OP * C, 128], [1, OP * C]]))

        # cm generation: 128 MMs + 128 biased drains. One MM per PSUM bank;
        # consecutive drains alternate DVE/Act across different banks (same-
        # bank DVE+Act reads would serialize). The previous d-half's proj
        # matmuls interleave into the gen stream to fill PE gaps.
        for jl in range(64):
            st = w2st[jl // 8]
            if dq == 2 and jl % 2 == 0:
                emit_proj(0, jl // 2)
            d0 = dq * 64 + jl
            for ch in range(2):
                pc = ps_gen.tile([128, M], F32, tag="gen")
                nc.tensor.matmul(pc, st[:, jl % 8, 128 * ch:128 * (ch + 1)],
                                 hT, start=True, stop=True)
                bias = b2cm_sb[:, ch, d0:d0 + 1]
                if (2 * jl + ch) % 2 == 0:
                    nc.vector.tensor_scalar_add(out=cm_sb[:, ch, jl, :], in0=pc,
                                                scalar1=bias)
                else:
                    nc.scalar.activation(out=cm_sb[:, ch, jl, :], in_=pc,
                                         func=AF.Identity, bias=bias,
                                         scale=1.0)

        # stage the next quarter's w2 now; transfers overlap the mix phase
        if dq < 3:
            w2st_cur = stage_w2(dq + 1)

        # sm params for the S matrix are produced once, tucked after the
        # first quarter's gen so their drains don't block startup
        if dq == 0:
            emit_smgen()

        # mixing, 8 batch-pairs of 8 groups, software-pipelined 2 pairs deep:
        # cm_sb reads (mix1 ldweights) finish early in the mix phase so the
        # next quarter's generation + drains overlap the mix2 tail.
        SKEW = 4

        def after_mix2(p):
            # on dq3, once pairs 0..2+p are drained (m 0..127 covered by
            # pairs 0-3), interleave the first m-half of the final proj
            if dq == 3 and 3 <= p <= 6:
                for o in range(8 * (p - 3), 8 * (p - 2)):
                    emit_proj_half(o, 0)

        o1gs = {}
        for pi in range(8):
            o1gs[pi] = emit_mix1_pair(dq, pi)
            if pi >= SKEW:
                og = o1gs.pop(pi - SKEW)
                for half in range(2):
                    emit_mix2(dq, 2 * (pi - SKEW) + half, og, half)
                after_mix2(pi - SKEW)
        for pi in range(8 - SKEW, 8):
            og = o1gs.pop(pi)
            for half in range(2):
                emit_mix2(dq, 2 * pi + half, og, half)
            after_mix2(pi)

    # final d-half proj (dh=1): second m-half
    for o in range(OP):
        emit_proj_half(o, 1)

    # ---------------- epilogue ----------------
    # y is stored e-major ([e_low, et, m], bf16); the host transposes back
    for et in range(2):
        nc.scalar.activation(out=outE[:, et, :], in_=acc[:, et, :],
                             func=AF.Identity, bias=pjb_sb[:, et:et + 1],
                             scale=1.0)
    nc.sync.dma_start(out=_ap(d_y, 0, [[2 * M, 128], [1, 2 * M]]), in_=outE)


def legalize_sync_waits(nc, max_waits=1):
    """This walrus build accepts only one sync wait per instruction; move
    extras onto preceding same-engine NoOps."""
    ctr = 0
    for f in nc.m.functions:
        for bb in f.blocks:
            out, changed = [], False
            for inst in bb.instructions:
                si = inst.sync_info
                if si is not None and si.on_wait and len(si.on_wait) > max_waits:
                    waits = list(si.on_wait)
                    for w in waits[:-max_waits]:
                        ctr += 1
                        n = mybir.InstNoOp(name=f"lw_nop_{ctr}", ins=[], outs=[])
                        n.engine = inst.engine
                        n.sync_info = mybir.SyncInfo(on_update=[], on_wait=[w])
                        out.append(n)
                    inst.sync_info = mybir.SyncInfo(
                        on_update=list(si.on_update or []),
                        on_wait=waits[-max_waits:])
                    changed = True
                out.append(inst)
            if changed:
                bb.instructions = out
    return ctr


_CACHE = {}


def _get_nc():
    if "nc" not in _CACHE:
        nc = bass.Bass()
        build(nc)
        legalize_sync_waits(nc)
        _CACHE["nc"] = nc
    return _CACHE["nc"]


def _prep_shared(inputs):
    import ml_dtypes
    bf16 = ml_dtypes.bfloat16
    f32 = np.float32
    w2 = np.asarray(inputs["w2"], f32)
    w1 = np.asarray(inputs["w1"], f32)
    b2 = np.asarray(inputs["b2"], f32)
    m_beta = np.asarray(inputs["m_beta"], f32)
    s_beta = np.asarray(inputs["s_beta"], f32)
    proj_w = np.asarray(inputs["proj_w"], f32)
    proj_b = np.asarray(inputs["proj_b"], f32)
    sh = {}
    # w2cm[dq, k, j, c] = w2[c*256 + dq*64 + j, k]
    sh["w2cm"] = np.ascontiguousarray(
        w2[:CC].reshape(C, 4, 64, HID).transpose(1, 3, 2, 0)
    ).astype(bf16).reshape(4, 128, 64 * C)
    sh["w2sm"] = np.ascontiguousarray(w2[CC:].T).astype(bf16)
    sh["w1t"] = np.ascontiguousarray(
        w1.reshape(HID, 2, 128).transpose(1, 2, 0)).astype(bf16)
    sh["b1v"] = np.asarray(inputs["b1"], f32)
    sh["b2cmv"] = np.ascontiguousarray(
        b2[:CC].reshape(2, 128, C).transpose(1, 0, 2)).reshape(128, 2 * C)
    sh["b2smv"] = np.ascontiguousarray(b2[CC:].reshape(8, 128).T)
    sh["ln_w"] = np.asarray(inputs["ln_w"], f32)
    sh["ln_b"] = np.asarray(inputs["ln_b"], f32)
    # mbq[dl + 64*h, dq] = m_beta[dq*64 + dl]
    mb = m_beta.reshape(4, 64).T  # [64, 4]
    sh["mbq"] = np.ascontiguousarray(np.concatenate([mb, mb], axis=0))
    sh["sbrow"] = np.tile(s_beta, 16).astype(bf16)
    # pwt[dh, f, o, e] = proj_w[e, o*256 + dh*128 + f]
    sh["pwt"] = np.ascontiguousarray(
        proj_w.reshape(C, OP, 2, 128).transpose(2, 3, 1, 0)
    ).astype(bf16).reshape(2, 128, OP * C)
    sh["pjb"] = np.ascontiguousarray(proj_b.reshape(2, 128).T)
    return sh


def kernel(**inputs):
    import ml_dtypes
    bf16 = ml_dtypes.bfloat16
    nc = _get_nc()
    x = np.asarray(inputs["x"], np.float32).reshape(B * N, P, C)
    query = np.asarray(inputs["query"], np.float32).reshape(B * N, C)
    # xh[ch, cl, m, p] = x[m, p, ch*128 + cl]
    xh = np.ascontiguousarray(
        x.reshape(B * N, P, 2, 128).transpose(2, 3, 0, 1)).astype(bf16)
    shared = _prep_shared(inputs)
    in_maps = []
    for c in range(NCORES):
        mmap = dict(shared)
        mmap["xh"] = np.ascontiguousarray(
            xh[:, :, c * M:(c + 1) * M, :]).reshape(2, 128, M * P)
        mmap["query"] = np.ascontiguousarray(query[c * M:(c + 1) * M])
        in_maps.append(mmap)
    res = run_bass_kernel_spmd(nc, in_maps, core_ids=list(range(NCORES)))
    outs = []
    for c in range(NCORES):
        ye = np.asarray(res.results[c]["y"]).reshape(128, 2, M)
        outs.append(ye.transpose(2, 1, 0).reshape(M, C).astype(np.float32))
    return np.concatenate(outs, axis=0).reshape(B, N, C)


if __name__ == "__main__":
    rng = np.random.default_rng(0)
    ins = {
        "x": rng.standard_normal((B, N, 1, P, C), dtype=np.float32),
        "query": rng.standard_normal((B, N, C), dtype=np.float32),
        "ln_w": np.full((C,), C ** -0.5, np.float32),
        "ln_b": np.zeros((C,), np.float32),
        "w1": (rng.standard_normal((HID, C)) * 0.02).astype(np.float32),
        "b1": np.zeros((HID,), np.float32),
        "w2": (rng.standard_normal((TOTAL, HID)) * 0.02).astype(np.float32),
        "b2": (rng.standard_normal((TOTAL,)) * 0.05).astype(np.float32),
        "m_beta": np.zeros((C,), np.float32),
        "s_beta": np.zeros((OP,), np.float32),
        "proj_w": (rng.standard_normal((C, OP * C)) * 0.02).astype(np.float32),
        "proj_b": np.zeros((C,), np.float32),
    }
    out = kernel(**ins)
    print("ran", out.shape, out.dtype)


# revision 52
# speedup vs baseline: 1.0038x; 1.0018x over previous
"""AdaptiveMixing Trainium2 kernel — 8-core data parallel, v2.

Per query n (M=256 per core):
  q  = LayerNorm(query[n]) * ln_w + ln_b
  h  = q @ w1.T + b1                      # [128]
  params = h @ w2.T + b2                  # [66560]
  cm = params[:65536].reshape(256, 256)
  sm = params[65536:].reshape(32, 32)
  o1 = gelu(x[n] @ cm + m_beta)           # [32, 256]
  o2 = gelu(sm @ o1 + s_beta[:, None])    # [32, 256]
  out[n] = o2.reshape(8192) @ proj_w.T + proj_b

Design notes (v2, ~4.7x over the v1 baseline):
- All weight transposes happen on the host: w2 arrives as [dq][k][j][c]
  bf16, x as [ch][c][m][p] bf16, proj_w as [dh][f][o][e] bf16, w1 as w1^T.
  This removes ~1500 on-device PE transposes + PSUM drains and halves the
  w2 HBM traffic (34 MB -> 17 MB per core).
- cm generation: 512 bf16 matmuls (w2 slab stationary, persistent hT
  moving, N=256), each drained PSUM->SBUF by a fused bias+cast
  tensor_scalar/activation, alternating DVE/Act across different banks
  (same-bank DVE+Act reads serialize on TRN2). 4 PSUM banks keep the
  drains back-to-back on both engines.
- Mixing-1 is flipped (cm as stationary, xs^T as moving) so d lands on
  partitions: m_beta folds into the gelu1 per-partition bias and two
  4-group batches share one [128, 512] PSUM bank via tile_position
  col-offset, halving gelu1 instruction count.
- Mixing-2 is merged with the transpose back to [d, (q, op)]: one matmul
  per 4-query group with the block-diagonal sm^T as the MOVING operand
  (stationary = gelu(o1)), s_beta injected by a K=1 ones-row matmul that
  opens the PSUM accumulation group.
- proj accumulates into one held PSUM bank over 64 f-chunks; the dh0 half
  interleaves into quarter-2's generation stream, the dh1 first m-half
  into quarter-3's mix tail.
- The mix phase is software-pipelined 2 batch-pairs deep so cm_sb reads
  retire early enough for the next quarter's generation to overlap.
"""

import sys

sys.path.insert(0, "/opt/trn_rl_repo")

import numpy as np

import concourse.bass as bass
import concourse.mybir as mybir
import concourse.tile as tile
from concourse.bass_utils import run_bass_kernel_spmd
from concourse.masks import make_identity

F32 = mybir.dt.float32
BF16 = mybir.dt.bfloat16
AF = mybir.ActivationFunctionType

B, N, P, C = 2, 1024, 32, 256
OP, HID = 32, 128
CC = C * C
TOTAL = CC + OP * P  # 66560
NCORES = 8
M = (B * N) // NCORES  # 256 queries per core
NG = M // 4            # 64 groups of 4 queries


def _ap(handle, offset, ap):
    return bass.AP(tensor=handle.ap().tensor, offset=offset, ap=[list(p) for p in ap])


def build(nc: bass.Bass):
    d_query = nc.dram_tensor("query", [M, C], F32, kind="ExternalInput")
    d_xh = nc.dram_tensor("xh", [2, 128, M * P], BF16, kind="ExternalInput")
    d_w2cm = nc.dram_tensor("w2cm", [4, 128, 64 * C], BF16, kind="ExternalInput")
    d_w2sm = nc.dram_tensor("w2sm", [128, 1024], BF16, kind="ExternalInput")
    d_w1t = nc.dram_tensor("w1t", [2, 128, 128], BF16, kind="ExternalInput")
    d_b1 = nc.dram_tensor("b1v", [HID], F32, kind="ExternalInput")
    d_b2cm = nc.dram_tensor("b2cmv", [128, 2 * C], F32, kind="ExternalInput")
    d_b2sm = nc.dram_tensor("b2smv", [128, 8], F32, kind="ExternalInput")
    d_lnw = nc.dram_tensor("ln_w", [C], F32, kind="ExternalInput")
    d_lnb = nc.dram_tensor("ln_b", [C], F32, kind="ExternalInput")
    d_mbq = nc.dram_tensor("mbq", [128, 4], F32, kind="ExternalInput")
    d_sbr = nc.dram_tensor("sbrow", [512], BF16, kind="ExternalInput")
    d_pwt = nc.dram_tensor("pwt", [2, 128, OP * C], BF16, kind="ExternalInput")
    d_pjb = nc.dram_tensor("pjb", [128, 2], F32, kind="ExternalInput")
    d_y = nc.dram_tensor("y", [128, 2 * M], BF16, kind="ExternalOutput")

    from contextlib import ExitStack
    with tile.TileContext(nc) as tc, ExitStack() as ctx:
        _body(ctx, nc, tc, d_query, d_xh, d_w2cm, d_w2sm, d_w1t, d_b1, d_b2cm,
              d_b2sm, d_lnw, d_lnb, d_mbq, d_sbr, d_pwt, d_pjb, d_y)
    return nc


def _body(ctx, nc, tc, d_query, d_xh, d_w2cm, d_w2sm, d_w1t, d_b1, d_b2cm,
          d_b2sm, d_lnw, d_lnb, d_mbq, d_sbr, d_pwt, d_pjb, d_y):
    singles = ctx.enter_context(tc.tile_pool(name="singles", bufs=1))
    tmp = ctx.enter_context(tc.tile_pool(name="tmp", bufs=4))
    w2st_p = ctx.enter_context(tc.tile_pool(name="w2st", bufs=8))
    o1g_p = ctx.enter_context(tc.tile_pool(name="o1g", bufs=8))
    m1g_p = ctx.enter_context(tc.tile_pool(name="m1g", bufs=4))
    ps_gen = ctx.enter_context(tc.tile_pool(name="ps_gen", bufs=4, space="PSUM"))
    ps_mix = ctx.enter_context(tc.tile_pool(name="ps_mix", bufs=3, space="PSUM"))
    ps_out = ctx.enter_context(tc.tile_pool(name="ps_out", bufs=1, space="PSUM"))

    # ---------------- constants / small DMAs ----------------
    ident_f = singles.tile([128, 128], F32)
    make_identity(nc, ident_f)
    ident_bf = singles.tile([128, 128], BF16)
    nc.vector.tensor_copy(out=ident_bf, in_=ident_f)

    # query first: the LayerNorm -> hT chain gates all cm generation
    qts, mvs, rstds = [], [], []
    for mt in range(2):
        qt = tmp.tile([128, C], F32, tag=f"qt{mt}")
        nc.sync.dma_start(out=qt, in_=_ap(d_query, mt * 128 * C, [[C, 128], [1, C]]))
        qts.append(qt)

    lnw_b = singles.tile([128, C], F32)
    nc.sync.dma_start(out=lnw_b, in_=_ap(d_lnw, 0, [[0, 128], [1, C]]))
    lnb_b = singles.tile([128, C], F32)
    nc.sync.dma_start(out=lnb_b, in_=_ap(d_lnb, 0, [[0, 128], [1, C]]))
    b1_sb = singles.tile([128, 1], F32)
    nc.sync.dma_start(out=b1_sb, in_=_ap(d_b1, 0, [[1, 128], [0, 1]]))
    b2cm_sb = singles.tile([128, 2, C], F32)
    nc.sync.dma_start(out=b2cm_sb, in_=_ap(d_b2cm, 0, [[2 * C, 128], [1, 2 * C]]))
    ones_sb = singles.tile([1, 64], BF16)
    nc.vector.memset(ones_sb, 1.0)
    eps_sb = singles.tile([128, 1], F32)
    nc.vector.memset(eps_sb, 1e-6)
    w1t_sb = singles.tile([128, 2, 128], BF16)
    nc.sync.dma_start(out=w1t_sb, in_=_ap(
        d_w1t, 0, [[128, 128], [128 * 128, 2], [1, 128]]))

    # ---------------- big persistent buffers ----------------
    cm_sb = singles.tile([128, 2, 64, M], BF16)     # [c_low, ch, j, m]
    xh_sb = singles.tile([128, 2, M, P], BF16)      # [c_low, ch, m, p]
    S_sb = singles.tile([128, NG, 128], BF16)       # [(q,p), g, (r,o)]
    flat2 = singles.tile([128, OP, M], BF16)        # [(parity,dl), o, m]
    pw_sb = singles.tile([128, OP, C], BF16)        # [f, o, e] one d-half
    qn_bf = singles.tile([128, 2, C], BF16)
    qnT = singles.tile([128, 2, M], BF16)
    hT = singles.tile([128, M], BF16)
    outE = singles.tile([128, 2, M], BF16)

    def stage_w2(dq):
        tiles = []
        for h in range(8):
            st = w2st_p.tile([128, 8, C], BF16, tag="w2st")
            nc.sync.dma_start(out=st, in_=_ap(
                d_w2cm, dq * 128 * 64 * C + h * 8 * C,
                [[64 * C, 128], [1, 8 * C]]))
            tiles.append(st)
        return tiles

    # quarter-0 w2 must beat w2sm and the big xh transfer into the queue
    w2st_cur = stage_w2(0)
    w2sm_sb = singles.tile([128, 1024], BF16)
    nc.sync.dma_start(out=w2sm_sb, in_=_ap(d_w2sm, 0, [[1024, 128], [1, 1024]]))
    b2sm_sb = singles.tile([128, 8], F32)
    nc.sync.dma_start(out=b2sm_sb, in_=_ap(d_b2sm, 0, [[8, 128], [1, 8]]))
    mbq_sb = singles.tile([128, 4], F32)
    nc.sync.dma_start(out=mbq_sb, in_=_ap(d_mbq, 0, [[4, 128], [1, 4]]))
    sbr_sb = singles.tile([1, 512], BF16)
    nc.sync.dma_start(out=sbr_sb, in_=_ap(d_sbr, 0, [[0, 1], [1, 512]]))
    pjb_sb = singles.tile([128, 2], F32)
    nc.sync.dma_start(out=pjb_sb, in_=_ap(d_pjb, 0, [[2, 128], [1, 2]]))
    for ch in range(2):
        nc.sync.dma_start(out=xh_sb[:, ch, :, :], in_=_ap(
            d_xh, ch * 128 * M * P, [[M * P, 128], [1, M * P]]))

    nc.gpsimd.memset(S_sb, 0.0)

    # ---------------- LayerNorm -> qn (bf16) ----------------
    for mt in range(2):
        stats = tmp.tile([128, 6], F32, tag="st")
        nc.vector.bn_stats(out=stats, in_=qts[mt])
        mv = tmp.tile([128, 2], F32, tag=f"mv{mt}")
        nc.vector.bn_aggr(out=mv, in_=stats)
        mvs.append(mv)
    for mt in range(2):
        rstd = tmp.tile([128, 1], F32, tag=f"rs{mt}")
        nc.scalar.activation(out=rstd, in_=mvs[mt][:, 1:2], func=AF.Sqrt,
                             bias=eps_sb, scale=1.0)
        nc.vector.reciprocal(out=rstd, in_=rstd)
        rstds.append(rstd)
    for mt in range(2):
        qt = qts[mt]
        nc.vector.tensor_scalar(out=qt, in0=qt, scalar1=mvs[mt][:, 0:1],
                                scalar2=rstds[mt],
                                op0=mybir.AluOpType.subtract,
                                op1=mybir.AluOpType.mult)
        nc.vector.tensor_mul(out=qt, in0=qt, in1=lnw_b)
        nc.vector.tensor_add(out=qn_bf[:, mt, :], in0=qt, in1=lnb_b)

    # qnT [c_low, ch, m]
    for mt in range(2):
        for ch in range(2):
            pt = ps_mix.tile([128, 2, 64], BF16, tag="mixps")
            ptv = pt.rearrange("p a b -> p (a b)")
            nc.tensor.transpose(ptv, qn_bf[:, mt, 128 * ch:128 * (ch + 1)], ident_bf)
            nc.vector.tensor_copy(out=qnT[:, ch, 128 * mt:128 * (mt + 1)], in_=ptv)

    # hT [k, m] = w1 @ qn.T + b1
    ps_h = ps_gen.tile([128, M], F32, tag="gen")
    for ch in range(2):
        nc.tensor.matmul(ps_h, w1t_sb[:, ch, :], qnT[:, ch, :],
                         start=(ch == 0), stop=(ch == 1))
    nc.vector.tensor_scalar_add(out=hT, in0=ps_h, scalar1=b1_sb)

    # ---------------- sm params -> S (block-diag smT + b2) ----------------
    def emit_smgen():
        for c4 in range(8):
            pg = ps_gen.tile([128, M], F32, tag="gen")
            nc.tensor.matmul(pg, w2sm_sb[:, 128 * c4:128 * (c4 + 1)], hT,
                             start=True, stop=True)
            for oi in range(4):
                op = 4 * c4 + oi
                for r in range(4):
                    src = pg[32 * oi:32 * oi + 32, :].rearrange(
                        "p (g r) -> p g r", r=4)
                    dst = S_sb[32 * r:32 * r + 32, :, 32 * r + op]
                    bias = b2sm_sb[32 * oi:32 * oi + 32, c4:c4 + 1]
                    if (oi + r) % 2 == 0:
                        nc.vector.tensor_scalar_add(out=dst, in0=src[:, :, r],
                                                    scalar1=bias)
                    else:
                        nc.scalar.activation(out=dst, in_=src[:, :, r],
                                             func=AF.Identity, bias=bias,
                                             scale=1.0)

    # ---------------- main loop over d-quarters ----------------
    acc = ps_out.tile([128, 2, M], F32)  # [e_low, et, m], held across loop

    def emit_mix1_pair(dq, pi):
        """64 matmuls for batch pair (2pi, 2pi+1) into one [128, 512] bank
        (odd batch lands on rows 64-127 via tile_position col-offset), then a
        single full-width gelu1 -> o1g [128, 4, 128]."""
        pm1 = ps_mix.tile([128, 512], F32, tag="mixps")
        for half in range(2):
            bi = 2 * pi + half
            r0 = 64 * half
            tp_pos = (0, 64) if half else (0, 0)
            for gi in range(4):
                g = 4 * bi + gi
                for jq in range(4):
                    m = 4 * g + jq
                    col = 128 * gi + 32 * jq
                    for ch in range(2):
                        first = (gi == 0 and jq == 0 and ch == 0)
                        last = (gi == 3 and jq == 3 and ch == 1)
                        nc.tensor.matmul(
                            pm1[r0:r0 + 64, col:col + 32],
                            cm_sb[:, ch, :, m], xh_sb[:, ch, m, :],
                            start=first, stop=last, tile_position=tp_pos)
        o1g = o1g_p.tile([128, 4, 128], BF16, tag="o1g")
        nc.scalar.activation(out=o1g, in_=pm1, func=AF.Gelu,
                             bias=mbq_sb[:, dq:dq + 1], scale=1.0)
        return o1g

    def emit_mix2(dq, bi, o1g, half):
        """Transposes, mix2 (merged with transpose) and gelu2 -> flat2."""
        parity = dq % 2
        r0 = 64 * half
        idn = ident_bf[r0:r0 + 64, r0:r0 + 64]
        m1g = m1g_p.tile([128, 4, 64], BF16, tag="m1g")
        pt = ps_mix.tile([128, 4, 64], BF16, tag="mixps")
        for i in range(4):
            nc.tensor.transpose(pt[:, i, :], o1g[r0:r0 + 64, i, :], idn)
        nc.vector.tensor_copy(out=m1g, in_=pt)
        o2p = ps_gen.tile([64, 512], F32, tag="gen")
        nc.tensor.matmul(o2p, ones_sb, sbr_sb, start=True, stop=False)
        for gi in range(4):
            g = 4 * bi + gi
            nc.tensor.matmul(o2p[:, 128 * gi:128 * (gi + 1)],
                             m1g[:, gi, :], S_sb[:, g, :],
                             start=False, stop=(gi == 3))
        dst = flat2[64 * parity:64 * parity + 64, :, 16 * bi:16 * bi + 16]
        dst = dst.rearrange("d o m -> d m o")
        nc.scalar.activation(out=dst, in_=o2p, func=AF.Gelu, scale=1.0)

    def emit_proj(dh, o):
        for et in range(2):
            nc.tensor.matmul(
                acc[:, et, :], pw_sb[:, o, 128 * et:128 * (et + 1)],
                flat2[:, o, :],
                start=(dh == 0 and o == 0 and et == 0),
                stop=False)

    def emit_proj_half(o, mh):
        for et in range(2):
            nc.tensor.matmul(
                acc[:, et, 128 * mh:128 * (mh + 1)],
                pw_sb[:, o, 128 * et:128 * (et + 1)],
                flat2[:, o, 128 * mh:128 * (mh + 1)],
                start=False,
                stop=(o == OP - 1 and et == 1 and mh == 1))

    for dq in range(4):
        parity = dq % 2
        w2st = w2st_cur
        # prefetch proj weights for this d-half at start of odd quarters
        if parity == 1:
            dh = dq // 2
            nc.sync.dma_start(out=pw_sb, in_=_ap(
                d_pwt, dh * 128 * OP * C, [[OP * C, 128], [1, OP * C]]))

        # cm generation: 128 MMs + 128 biased drains. One MM per PSUM bank;
        # consecutive drains alternate DVE/Act across different banks (same-
        # bank DVE+Act reads would serialize). The previous d-half's proj
        # matmuls interleave into the gen stream to fill PE gaps.
        for jl in range(64):
            st = w2st[jl // 8]
            if dq == 2 and jl % 2 == 0:
                emit_proj(0, jl // 2)
            d0 = dq * 64 + jl
            for ch in range(2):
                pc = ps_gen.tile([128, M], F32, tag="gen")
                nc.tensor.matmul(pc, st[:, jl % 8, 128 * ch:128 * (ch + 1)],
                                 hT, start=True, stop=True)
                bias = b2cm_sb[:, ch, d0:d0 + 1]
                if (2 * jl + ch) % 2 == 0:
                    nc.vector.tensor_scalar_add(out=cm_sb[:, ch, jl, :], in0=pc,
                                                scalar1=bias)
                else:
                    nc.scalar.activation(out=cm_sb[:, ch, jl, :], in_=pc,
                                         func=AF.Identity, bias=bias,
                                         scale=1.0)

        # stage the next quarter's w2 now; transfers overlap the mix phase
        if dq < 3:
            w2st_cur = stage_w2(dq + 1)

        # sm params for the S matrix are produced once, tucked after the
        # first quarter's gen so their drains don't block startup
        if dq == 0:
            emit_smgen()

        # mixing, 8 batch-pairs of 8 groups, software-pipelined 2 pairs deep:
        # cm_sb reads (mix1 ldweights) finish early in the mix phase so the
        # next quarter's generation + drains overlap the mix2 tail.
        SKEW = 4 if dq < 3 else 2

        def after_mix2(p):
            # on dq3, once pairs 0..2+p are drained (m 0..127 covered by
            # pairs 0-3), interleave the first m-half of the final proj
            if dq == 3 and 3 <= p <= 6:
                for o in range(8 * (p - 3), 8 * (p - 2)):
                    emit_proj_half(o, 0)

        o1gs = {}
        for pi in range(8):
            o1gs[pi] = emit_mix1_pair(dq, pi)
            if pi >= SKEW:
                og = o1gs.pop(pi - SKEW)
                for half in range(2):
                    emit_mix2(dq, 2 * (pi - SKEW) + half, og, half)
                after_mix2(pi - SKEW)
        for pi in range(8 - SKEW, 8):
            og = o1gs.pop(pi)
            for half in range(2):
                emit_mix2(dq, 2 * pi + half, og, half)
            after_mix2(pi)

    # final d-half proj (dh=1): second m-half
    for o in range(OP):
        emit_proj_half(o, 1)

    # ---------------- epilogue ----------------
    # y is stored e-major ([e_low, et, m], bf16); the host transposes back
    for et in range(2):
        nc.scalar.activation(out=outE[:, et, :], in_=acc[:, et, :],
                             func=AF.Identity, bias=pjb_sb[:, et:et + 1],
                             scale=1.0)
    nc.sync.dma_start(out=_ap(d_y, 0, [[2 * M, 128], [1, 2 * M]]), in_=outE)


def legalize_sync_waits(nc, max_waits=1):
    """This walrus build accepts only one sync wait per instruction; move
    extras onto preceding same-engine NoOps."""
    ctr = 0
    for f in nc.m.functions:
        for bb in f.blocks:
            out, changed = [], False
            for inst in bb.instructions:
                si = inst.sync_info
                if si is not None and si.on_wait and len(si.on_wait) > max_waits:
                    waits = list(si.on_wait)
                    for w in waits[:-max_waits]:
                        ctr += 1
                        n = mybir.InstNoOp(name=f"lw_nop_{ctr}", ins=[], outs=[])
                        n.engine = inst.engine
                        n.sync_info = mybir.SyncInfo(on_update=[], on_wait=[w])
                        out.append(n)
                    inst.sync_info = mybir.SyncInfo(
                        on_update=list(si.on_update or []),
                        on_wait=waits[-max_waits:])
                    changed = True
                out.append(inst)
            if changed:
                bb.instructions = out
    return ctr


_CACHE = {}


def _get_nc():
    if "nc" not in _CACHE:
        nc = bass.Bass()
        build(nc)
        legalize_sync_waits(nc)
        _CACHE["nc"] = nc
    return _CACHE["nc"]


def _prep_shared(inputs):
    import ml_dtypes
    bf16 = ml_dtypes.bfloat16
    f32 = np.float32
    w2 = np.asarray(inputs["w2"], f32)
    w1 = np.asarray(inputs["w1"], f32)
    b2 = np.asarray(inputs["b2"], f32)
    m_beta = np.asarray(inputs["m_beta"], f32)
    s_beta = np.asarray(inputs["s_beta"], f32)
    proj_w = np.asarray(inputs["proj_w"], f32)
    proj_b = np.asarray(inputs["proj_b"], f32)
    sh = {}
    # w2cm[dq, k, j, c] = w2[c*256 + dq*64 + j, k]
    sh["w2cm"] = np.ascontiguousarray(
        w2[:CC].reshape(C, 4, 64, HID).transpose(1, 3, 2, 0)
    ).astype(bf16).reshape(4, 128, 64 * C)
    sh["w2sm"] = np.ascontiguousarray(w2[CC:].T).astype(bf16)
    sh["w1t"] = np.ascontiguousarray(
        w1.reshape(HID, 2, 128).transpose(1, 2, 0)).astype(bf16)
    sh["b1v"] = np.asarray(inputs["b1"], f32)
    sh["b2cmv"] = np.ascontiguousarray(
        b2[:CC].reshape(2, 128, C).transpose(1, 0, 2)).reshape(128, 2 * C)
    sh["b2smv"] = np.ascontiguousarray(b2[CC:].reshape(8, 128).T)
    sh["ln_w"] = np.asarray(inputs["ln_w"], f32)
    sh["ln_b"] = np.asarray(inputs["ln_b"], f32)
    # mbq[dl + 64*h, dq] = m_beta[dq*64 + dl]
    mb = m_beta.reshape(4, 64).T  # [64, 4]
    sh["mbq"] = np.ascontiguousarray(np.concatenate([mb, mb], axis=0))
    sh["sbrow"] = np.tile(s_beta, 16).astype(bf16)
    # pwt[dh, f, o, e] = proj_w[e, o*256 + dh*128 + f]
    sh["pwt"] = np.ascontiguousarray(
        proj_w.reshape(C, OP, 2, 128).transpose(2, 3, 1, 0)
    ).astype(bf16).reshape(2, 128, OP * C)
    sh["pjb"] = np.ascontiguousarray(proj_b.reshape(2, 128).T)
    return sh


def kernel(**inputs):
    import ml_dtypes
    bf16 = ml_dtypes.bfloat16
    nc = _get_nc()
    x = np.asarray(inputs["x"], np.float32).reshape(B * N, P, C)
    query = np.asarray(inputs["query"], np.float32).reshape(B * N, C)
    # xh[ch, cl, m, p] = x[m, p, ch*128 + cl]
    xh = np.ascontiguousarray(
        x.reshape(B * N, P, 2, 128).transpose(2, 3, 0, 1)).astype(bf16)
    shared = _prep_shared(inputs)
    in_maps = []
    for c in range(NCORES):
        mmap = dict(shared)
        mmap["xh"] = np.ascontiguousarray(
            xh[:, :, c * M:(c + 1) * M, :]).reshape(2, 128, M * P)
        mmap["query"] = np.ascontiguousarray(query[c * M:(c + 1) * M])
        in_maps.append(mmap)
    res = run_bass_kernel_spmd(nc, in_maps, core_ids=list(range(NCORES)))
    outs = []
    for c in range(NCORES):
        ye = np.asarray(res.results[c]["y"]).reshape(128, 2, M)
        outs.append(ye.transpose(2, 1, 0).reshape(M, C).astype(np.float32))
    return np.concatenate(outs, axis=0).reshape(B, N, C)


if __name__ == "__main__":
    rng = np.random.default_rng(0)
    ins = {
        "x": rng.standard_normal((B, N, 1, P, C), dtype=np.float32),
        "query": rng.standard_normal((B, N, C), dtype=np.float32),
        "ln_w": np.full((C,), C ** -0.5, np.float32),
        "ln_b": np.zeros((C,), np.float32),
        "w1": (rng.standard_normal((HID, C)) * 0.02).astype(np.float32),
        "b1": np.zeros((HID,), np.float32),
        "w2": (rng.standard_normal((TOTAL, HID)) * 0.02).astype(np.float32),
        "b2": (rng.standard_normal((TOTAL,)) * 0.05).astype(np.float32),
        "m_beta": np.zeros((C,), np.float32),
        "s_beta": np.zeros((OP,), np.float32),
        "proj_w": (rng.standard_normal((C, OP * C)) * 0.02).astype(np.float32),
        "proj_b": np.zeros((C,), np.float32),
    }
    out = kernel(**ins)
    print("ran", out.shape, out.dtype)


# revision 53
# speedup vs baseline: 1.0054x; 1.0016x over previous
"""AdaptiveMixing Trainium2 kernel — 8-core data parallel, v2.

Per query n (M=256 per core):
  q  = LayerNorm(query[n]) * ln_w + ln_b
  h  = q @ w1.T + b1                      # [128]
  params = h @ w2.T + b2                  # [66560]
  cm = params[:65536].reshape(256, 256)
  sm = params[65536:].reshape(32, 32)
  o1 = gelu(x[n] @ cm + m_beta)           # [32, 256]
  o2 = gelu(sm @ o1 + s_beta[:, None])    # [32, 256]
  out[n] = o2.reshape(8192) @ proj_w.T + proj_b

Design notes (v2, ~4.7x over the v1 baseline):
- All weight transposes happen on the host: w2 arrives as [dq][k][j][c]
  bf16, x as [ch][c][m][p] bf16, proj_w as [dh][f][o][e] bf16, w1 as w1^T.
  This removes ~1500 on-device PE transposes + PSUM drains and halves the
  w2 HBM traffic (34 MB -> 17 MB per core).
- cm generation: 512 bf16 matmuls (w2 slab stationary, persistent hT
  moving, N=256), each drained PSUM->SBUF by a fused bias+cast
  tensor_scalar/activation, alternating DVE/Act across different banks
  (same-bank DVE+Act reads serialize on TRN2). 4 PSUM banks keep the
  drains back-to-back on both engines.
- Mixing-1 is flipped (cm as stationary, xs^T as moving) so d lands on
  partitions: m_beta folds into the gelu1 per-partition bias and two
  4-group batches share one [128, 512] PSUM bank via tile_position
  col-offset, halving gelu1 instruction count.
- Mixing-2 is merged with the transpose back to [d, (q, op)]: one matmul
  per 4-query group with the block-diagonal sm^T as the MOVING operand
  (stationary = gelu(o1)), s_beta injected by a K=1 ones-row matmul that
  opens the PSUM accumulation group.
- proj accumulates into one held PSUM bank over 64 f-chunks; the dh0 half
  interleaves into quarter-2's generation stream, the dh1 first m-half
  into quarter-3's mix tail.
- The mix phase is software-pipelined 2 batch-pairs deep so cm_sb reads
  retire early enough for the next quarter's generation to overlap.
"""

import sys

sys.path.insert(0, "/opt/trn_rl_repo")

import numpy as np

import concourse.bass as bass
import concourse.mybir as mybir
import concourse.tile as tile
from concourse.bass_utils import run_bass_kernel_spmd
from concourse.masks import make_identity

F32 = mybir.dt.float32
BF16 = mybir.dt.bfloat16
AF = mybir.ActivationFunctionType

B, N, P, C = 2, 1024, 32, 256
OP, HID = 32, 128
CC = C * C
TOTAL = CC + OP * P  # 66560
NCORES = 8
M = (B * N) // NCORES  # 256 queries per core
NG = M // 4            # 64 groups of 4 queries


def _ap(handle, offset, ap):
    return bass.AP(tensor=handle.ap().tensor, offset=offset, ap=[list(p) for p in ap])


def build(nc: bass.Bass):
    d_query = nc.dram_tensor("query", [M, C], F32, kind="ExternalInput")
    d_xh = nc.dram_tensor("xh", [2, 128, M * P], BF16, kind="ExternalInput")
    d_w2cm = nc.dram_tensor("w2cm", [4, 128, 64 * C], BF16, kind="ExternalInput")
    d_w2sm = nc.dram_tensor("w2sm", [128, 1024], BF16, kind="ExternalInput")
    d_w1t = nc.dram_tensor("w1t", [2, 128, 128], BF16, kind="ExternalInput")
    d_b1 = nc.dram_tensor("b1v", [HID], F32, kind="ExternalInput")
    d_b2cm = nc.dram_tensor("b2cmv", [128, 2 * C], F32, kind="ExternalInput")
    d_b2sm = nc.dram_tensor("b2smv", [128, 8], F32, kind="ExternalInput")
    d_lnw = nc.dram_tensor("ln_w", [C], F32, kind="ExternalInput")
    d_lnb = nc.dram_tensor("ln_b", [C], F32, kind="ExternalInput")
    d_mbq = nc.dram_tensor("mbq", [128, 4], F32, kind="ExternalInput")
    d_sbr = nc.dram_tensor("sbrow", [512], BF16, kind="ExternalInput")
    d_pwt = nc.dram_tensor("pwt", [2, 128, OP * C], BF16, kind="ExternalInput")
    d_pjb = nc.dram_tensor("pjb", [128, 2], F32, kind="ExternalInput")
    d_y = nc.dram_tensor("y", [128, 2 * M], BF16, kind="ExternalOutput")

    from contextlib import ExitStack
    with tile.TileContext(nc) as tc, ExitStack() as ctx:
        _body(ctx, nc, tc, d_query, d_xh, d_w2cm, d_w2sm, d_w1t, d_b1, d_b2cm,
              d_b2sm, d_lnw, d_lnb, d_mbq, d_sbr, d_pwt, d_pjb, d_y)
    return nc


def _body(ctx, nc, tc, d_query, d_xh, d_w2cm, d_w2sm, d_w1t, d_b1, d_b2cm,
          d_b2sm, d_lnw, d_lnb, d_mbq, d_sbr, d_pwt, d_pjb, d_y):
    singles = ctx.enter_context(tc.tile_pool(name="singles", bufs=1))
    tmp = ctx.enter_context(tc.tile_pool(name="tmp", bufs=4))
    w2st_p = ctx.enter_context(tc.tile_pool(name="w2st", bufs=8))
    o1g_p = ctx.enter_context(tc.tile_pool(name="o1g", bufs=8))
    m1g_p = ctx.enter_context(tc.tile_pool(name="m1g", bufs=4))
    ps_gen = ctx.enter_context(tc.tile_pool(name="ps_gen", bufs=4, space="PSUM"))
    ps_mix = ctx.enter_context(tc.tile_pool(name="ps_mix", bufs=3, space="PSUM"))
    ps_out = ctx.enter_context(tc.tile_pool(name="ps_out", bufs=1, space="PSUM"))

    # ---------------- constants / small DMAs ----------------
    ident_f = singles.tile([128, 128], F32)
    make_identity(nc, ident_f)
    ident_bf = singles.tile([128, 128], BF16)
    nc.vector.tensor_copy(out=ident_bf, in_=ident_f)

    # query first: the LayerNorm -> hT chain gates all cm generation.
    # One DMA for both m-tiles: partition = m_low, free = (mt, c)
    qboth = tmp.tile([128, 2, C], F32, tag="qb")
    nc.sync.dma_start(out=qboth, in_=_ap(
        d_query, 0, [[C, 128], [128 * C, 2], [1, C]]))
    qts = [qboth[:, 0, :], qboth[:, 1, :]]
    mvs, rstds = [], []

    lnw_b = singles.tile([128, C], F32)
    nc.sync.dma_start(out=lnw_b, in_=_ap(d_lnw, 0, [[0, 128], [1, C]]))
    lnb_b = singles.tile([128, C], F32)
    nc.sync.dma_start(out=lnb_b, in_=_ap(d_lnb, 0, [[0, 128], [1, C]]))
    b1_sb = singles.tile([128, 1], F32)
    nc.sync.dma_start(out=b1_sb, in_=_ap(d_b1, 0, [[1, 128], [0, 1]]))
    b2cm_sb = singles.tile([128, 2, C], F32)
    nc.sync.dma_start(out=b2cm_sb, in_=_ap(d_b2cm, 0, [[2 * C, 128], [1, 2 * C]]))
    ones_sb = singles.tile([1, 64], BF16)
    nc.vector.memset(ones_sb, 1.0)
    eps_sb = singles.tile([128, 1], F32)
    nc.vector.memset(eps_sb, 1e-6)
    w1t_sb = singles.tile([128, 2, 128], BF16)
    nc.sync.dma_start(out=w1t_sb, in_=_ap(
        d_w1t, 0, [[128, 128], [128 * 128, 2], [1, 128]]))

    # ---------------- big persistent buffers ----------------
    cm_sb = singles.tile([128, 2, 64, M], BF16)     # [c_low, ch, j, m]
    xh_sb = singles.tile([128, 2, M, P], BF16)      # [c_low, ch, m, p]
    S_sb = singles.tile([128, NG, 128], BF16)       # [(q,p), g, (r,o)]
    flat2 = singles.tile([128, OP, M], BF16)        # [(parity,dl), o, m]
    pw_sb = singles.tile([128, OP, C], BF16)        # [f, o, e] one d-half
    qn_bf = singles.tile([128, 2, C], BF16)
    qnT = singles.tile([128, 2, M], BF16)
    hT = singles.tile([128, M], BF16)
    outE = singles.tile([128, 2, M], BF16)

    def stage_w2(dq):
        tiles = []
        for h in range(8):
            st = w2st_p.tile([128, 8, C], BF16, tag="w2st")
            nc.sync.dma_start(out=st, in_=_ap(
                d_w2cm, dq * 128 * 64 * C + h * 8 * C,
                [[64 * C, 128], [1, 8 * C]]))
            tiles.append(st)
        return tiles

    # quarter-0 w2 must beat w2sm and the big xh transfer into the queue
    w2st_cur = stage_w2(0)
    w2sm_sb = singles.tile([128, 1024], BF16)
    nc.sync.dma_start(out=w2sm_sb, in_=_ap(d_w2sm, 0, [[1024, 128], [1, 1024]]))
    b2sm_sb = singles.tile([128, 8], F32)
    nc.sync.dma_start(out=b2sm_sb, in_=_ap(d_b2sm, 0, [[8, 128], [1, 8]]))
    mbq_sb = singles.tile([128, 4], F32)
    nc.sync.dma_start(out=mbq_sb, in_=_ap(d_mbq, 0, [[4, 128], [1, 4]]))
    sbr_sb = singles.tile([1, 512], BF16)
    nc.sync.dma_start(out=sbr_sb, in_=_ap(d_sbr, 0, [[0, 1], [1, 512]]))
    pjb_sb = singles.tile([128, 2], F32)
    nc.sync.dma_start(out=pjb_sb, in_=_ap(d_pjb, 0, [[2, 128], [1, 2]]))
    for ch in range(2):
        nc.sync.dma_start(out=xh_sb[:, ch, :, :], in_=_ap(
            d_xh, ch * 128 * M * P, [[M * P, 128], [1, M * P]]))

    nc.gpsimd.memset(S_sb, 0.0)

    # ---------------- LayerNorm -> qn (bf16) ----------------
    for mt in range(2):
        stats = tmp.tile([128, 6], F32, tag="st")
        nc.vector.bn_stats(out=stats, in_=qts[mt])
        mv = tmp.tile([128, 2], F32, tag=f"mv{mt}")
        nc.vector.bn_aggr(out=mv, in_=stats)
        mvs.append(mv)
    for mt in range(2):
        rstd = tmp.tile([128, 1], F32, tag=f"rs{mt}")
        nc.scalar.activation(out=rstd, in_=mvs[mt][:, 1:2], func=AF.Sqrt,
                             bias=eps_sb, scale=1.0)
        nc.vector.reciprocal(out=rstd, in_=rstd)
        rstds.append(rstd)
    for mt in range(2):
        qt = qts[mt]
        nc.vector.tensor_scalar(out=qt, in0=qt, scalar1=mvs[mt][:, 0:1],
                                scalar2=rstds[mt],
                                op0=mybir.AluOpType.subtract,
                                op1=mybir.AluOpType.mult)
        nc.vector.tensor_mul(out=qt, in0=qt, in1=lnw_b)
        nc.vector.tensor_add(out=qn_bf[:, mt, :], in0=qt, in1=lnb_b)

    # qnT [c_low, ch, m]
    for mt in range(2):
        for ch in range(2):
            pt = ps_mix.tile([128, 2, 64], BF16, tag="mixps")
            ptv = pt.rearrange("p a b -> p (a b)")
            nc.tensor.transpose(ptv, qn_bf[:, mt, 128 * ch:128 * (ch + 1)], ident_bf)
            nc.vector.tensor_copy(out=qnT[:, ch, 128 * mt:128 * (mt + 1)], in_=ptv)

    # hT [k, m] = w1 @ qn.T + b1
    ps_h = ps_gen.tile([128, M], F32, tag="gen")
    for ch in range(2):
        nc.tensor.matmul(ps_h, w1t_sb[:, ch, :], qnT[:, ch, :],
                         start=(ch == 0), stop=(ch == 1))
    nc.vector.tensor_scalar_add(out=hT, in0=ps_h, scalar1=b1_sb)

    # ---------------- sm params -> S (block-diag smT + b2) ----------------
    def emit_smgen():
        for c4 in range(8):
            pg = ps_gen.tile([128, M], F32, tag="gen")
            nc.tensor.matmul(pg, w2sm_sb[:, 128 * c4:128 * (c4 + 1)], hT,
                             start=True, stop=True)
            for oi in range(4):
                op = 4 * c4 + oi
                for r in range(4):
                    src = pg[32 * oi:32 * oi + 32, :].rearrange(
                        "p (g r) -> p g r", r=4)
                    dst = S_sb[32 * r:32 * r + 32, :, 32 * r + op]
                    bias = b2sm_sb[32 * oi:32 * oi + 32, c4:c4 + 1]
                    if (oi + r) % 2 == 0:
                        nc.vector.tensor_scalar_add(out=dst, in0=src[:, :, r],
                                                    scalar1=bias)
                    else:
                        nc.scalar.activation(out=dst, in_=src[:, :, r],
                                             func=AF.Identity, bias=bias,
                                             scale=1.0)

    # ---------------- main loop over d-quarters ----------------
    acc = ps_out.tile([128, 2, M], F32)  # [e_low, et, m], held across loop

    def emit_mix1_pair(dq, pi):
        """64 matmuls for batch pair (2pi, 2pi+1) into one [128, 512] bank
        (odd batch lands on rows 64-127 via tile_position col-offset), then a
        single full-width gelu1 -> o1g [128, 4, 128]."""
        pm1 = ps_mix.tile([128, 512], F32, tag="mixps")
        for half in range(2):
            bi = 2 * pi + half
            r0 = 64 * half
            tp_pos = (0, 64) if half else (0, 0)
            for gi in range(4):
                g = 4 * bi + gi
                for jq in range(4):
                    m = 4 * g + jq
                    col = 128 * gi + 32 * jq
                    for ch in range(2):
                        first = (gi == 0 and jq == 0 and ch == 0)
                        last = (gi == 3 and jq == 3 and ch == 1)
                        nc.tensor.matmul(
                            pm1[r0:r0 + 64, col:col + 32],
                            cm_sb[:, ch, :, m], xh_sb[:, ch, m, :],
                            start=first, stop=last, tile_position=tp_pos)
        o1g = o1g_p.tile([128, 4, 128], BF16, tag="o1g")
        nc.scalar.activation(out=o1g, in_=pm1, func=AF.Gelu,
                             bias=mbq_sb[:, dq:dq + 1], scale=1.0)
        return o1g

    def emit_mix2(dq, bi, o1g, half):
        """Transposes, mix2 (merged with transpose) and gelu2 -> flat2."""
        parity = dq % 2
        r0 = 64 * half
        idn = ident_bf[r0:r0 + 64, r0:r0 + 64]
        m1g = m1g_p.tile([128, 4, 64], BF16, tag="m1g")
        pt = ps_mix.tile([128, 4, 64], BF16, tag="mixps")
        for i in range(4):
            nc.tensor.transpose(pt[:, i, :], o1g[r0:r0 + 64, i, :], idn)
        nc.vector.tensor_copy(out=m1g, in_=pt)
        o2p = ps_gen.tile([64, 512], F32, tag="gen")
        nc.tensor.matmul(o2p, ones_sb, sbr_sb, start=True, stop=False)
        for gi in range(4):
            g = 4 * bi + gi
            nc.tensor.matmul(o2p[:, 128 * gi:128 * (gi + 1)],
                             m1g[:, gi, :], S_sb[:, g, :],
                             start=False, stop=(gi == 3))
        dst = flat2[64 * parity:64 * parity + 64, :, 16 * bi:16 * bi + 16]
        dst = dst.rearrange("d o m -> d m o")
        nc.scalar.activation(out=dst, in_=o2p, func=AF.Gelu, scale=1.0)

    def emit_proj(dh, o):
        for et in range(2):
            nc.tensor.matmul(
                acc[:, et, :], pw_sb[:, o, 128 * et:128 * (et + 1)],
                flat2[:, o, :],
                start=(dh == 0 and o == 0 and et == 0),
                stop=False)

    def emit_proj_half(o, mh):
        for et in range(2):
            nc.tensor.matmul(
                acc[:, et, 128 * mh:128 * (mh + 1)],
                pw_sb[:, o, 128 * et:128 * (et + 1)],
                flat2[:, o, 128 * mh:128 * (mh + 1)],
                start=False,
                stop=(o == OP - 1 and et == 1 and mh == 1))

    for dq in range(4):
        parity = dq % 2
        w2st = w2st_cur
        # prefetch proj weights for this d-half at start of odd quarters
        if parity == 1:
            dh = dq // 2
            nc.sync.dma_start(out=pw_sb, in_=_ap(
                d_pwt, dh * 128 * OP * C, [[OP * C, 128], [1, OP * C]]))

        # cm generation: 128 MMs + 128 biased drains. One MM per PSUM bank;
        # consecutive drains alternate DVE/Act across different banks (same-
        # bank DVE+Act reads would serialize). The previous d-half's proj
        # matmuls interleave into the gen stream to fill PE gaps.
        for jl in range(64):
            st = w2st[jl // 8]
            if dq == 2 and jl % 2 == 0:
                emit_proj(0, jl // 2)
            d0 = dq * 64 + jl
            for ch in range(2):
                pc = ps_gen.tile([128, M], F32, tag="gen")
                nc.tensor.matmul(pc, st[:, jl % 8, 128 * ch:128 * (ch + 1)],
                                 hT, start=True, stop=True)
                bias = b2cm_sb[:, ch, d0:d0 + 1]
                if (2 * jl + ch) % 2 == 0:
                    nc.vector.tensor_scalar_add(out=cm_sb[:, ch, jl, :], in0=pc,
                                                scalar1=bias)
                else:
                    nc.scalar.activation(out=cm_sb[:, ch, jl, :], in_=pc,
                                         func=AF.Identity, bias=bias,
                                         scale=1.0)

        # stage the next quarter's w2 now; transfers overlap the mix phase
        if dq < 3:
            w2st_cur = stage_w2(dq + 1)

        # sm params for the S matrix are produced once, tucked after the
        # first quarter's gen so their drains don't block startup
        if dq == 0:
            emit_smgen()

        # mixing, 8 batch-pairs of 8 groups, software-pipelined 2 pairs deep:
        # cm_sb reads (mix1 ldweights) finish early in the mix phase so the
        # next quarter's generation + drains overlap the mix2 tail.
        SKEW = 4 if dq < 3 else 2

        def after_mix2(p):
            # on dq3, once pairs 0..2+p are drained (m 0..127 covered by
            # pairs 0-3), interleave the first m-half of the final proj
            if dq == 3 and 3 <= p <= 6:
                for o in range(8 * (p - 3), 8 * (p - 2)):
                    emit_proj_half(o, 0)

        o1gs = {}
        for pi in range(8):
            o1gs[pi] = emit_mix1_pair(dq, pi)
            if pi >= SKEW:
                og = o1gs.pop(pi - SKEW)
                for half in range(2):
                    emit_mix2(dq, 2 * (pi - SKEW) + half, og, half)
                after_mix2(pi - SKEW)
        for pi in range(8 - SKEW, 8):
            og = o1gs.pop(pi)
            for half in range(2):
                emit_mix2(dq, 2 * pi + half, og, half)
            after_mix2(pi)

    # final d-half proj (dh=1): second m-half
    for o in range(OP):
        emit_proj_half(o, 1)

    # ---------------- epilogue ----------------
    # y is stored e-major ([e_low, et, m], bf16); the host transposes back
    for et in range(2):
        nc.scalar.activation(out=outE[:, et, :], in_=acc[:, et, :],
                             func=AF.Identity, bias=pjb_sb[:, et:et + 1],
                             scale=1.0)
    nc.sync.dma_start(out=_ap(d_y, 0, [[2 * M, 128], [1, 2 * M]]), in_=outE)


def legalize_sync_waits(nc, max_waits=1):
    """This walrus build accepts only one sync wait per instruction; move
    extras onto preceding same-engine NoOps."""
    ctr = 0
    for f in nc.m.functions:
        for bb in f.blocks:
            out, changed = [], False
            for inst in bb.instructions:
                si = inst.sync_info
                if si is not None and si.on_wait and len(si.on_wait) > max_waits:
                    waits = list(si.on_wait)
                    for w in waits[:-max_waits]:
                        ctr += 1
                        n = mybir.InstNoOp(name=f"lw_nop_{ctr}", ins=[], outs=[])
                        n.engine = inst.engine
                        n.sync_info = mybir.SyncInfo(on_update=[], on_wait=[w])
                        out.append(n)
                    inst.sync_info = mybir.SyncInfo(
                        on_update=list(si.on_update or []),
                        on_wait=waits[-max_waits:])
                    changed = True
                out.append(inst)
            if changed:
                bb.instructions = out
    return ctr


_CACHE = {}


def _get_nc():
    if "nc" not in _CACHE:
        nc = bass.Bass()
        build(nc)
        legalize_sync_waits(nc)
        _CACHE["nc"] = nc
    return _CACHE["nc"]


def _prep_shared(inputs):
    import ml_dtypes
    bf16 = ml_dtypes.bfloat16
    f32 = np.float32
    w2 = np.asarray(inputs["w2"], f32)
    w1 = np.asarray(inputs["w1"], f32)
    b2 = np.asarray(inputs["b2"], f32)
    m_beta = np.asarray(inputs["m_beta"], f32)
    s_beta = np.asarray(inputs["s_beta"], f32)
    proj_w = np.asarray(inputs["proj_w"], f32)
    proj_b = np.asarray(inputs["proj_b"], f32)
    sh = {}
    # w2cm[dq, k, j, c] = w2[c*256 + dq*64 + j, k]
    sh["w2cm"] = np.ascontiguousarray(
        w2[:CC].reshape(C, 4, 64, HID).transpose(1, 3, 2, 0)
    ).astype(bf16).reshape(4, 128, 64 * C)
    sh["w2sm"] = np.ascontiguousarray(w2[CC:].T).astype(bf16)
    sh["w1t"] = np.ascontiguousarray(
        w1.reshape(HID, 2, 128).transpose(1, 2, 0)).astype(bf16)
    sh["b1v"] = np.asarray(inputs["b1"], f32)
    sh["b2cmv"] = np.ascontiguousarray(
        b2[:CC].reshape(2, 128, C).transpose(1, 0, 2)).reshape(128, 2 * C)
    sh["b2smv"] = np.ascontiguousarray(b2[CC:].reshape(8, 128).T)
    sh["ln_w"] = np.asarray(inputs["ln_w"], f32)
    sh["ln_b"] = np.asarray(inputs["ln_b"], f32)
    # mbq[dl + 64*h, dq] = m_beta[dq*64 + dl]
    mb = m_beta.reshape(4, 64).T  # [64, 4]
    sh["mbq"] = np.ascontiguousarray(np.concatenate([mb, mb], axis=0))
    sh["sbrow"] = np.tile(s_beta, 16).astype(bf16)
    # pwt[dh, f, o, e] = proj_w[e, o*256 + dh*128 + f]
    sh["pwt"] = np.ascontiguousarray(
        proj_w.reshape(C, OP, 2, 128).transpose(2, 3, 1, 0)
    ).astype(bf16).reshape(2, 128, OP * C)
    sh["pjb"] = np.ascontiguousarray(proj_b.reshape(2, 128).T)
    return sh


def kernel(**inputs):
    import ml_dtypes
    bf16 = ml_dtypes.bfloat16
    nc = _get_nc()
    x = np.asarray(inputs["x"], np.float32).reshape(B * N, P, C)
    query = np.asarray(inputs["query"], np.float32).reshape(B * N, C)
    # xh[ch, cl, m, p] = x[m, p, ch*128 + cl]
    xh = np.ascontiguousarray(
        x.reshape(B * N, P, 2, 128).transpose(2, 3, 0, 1)).astype(bf16)
    shared = _prep_shared(inputs)
    in_maps = []
    for c in range(NCORES):
        mmap = dict(shared)
        mmap["xh"] = np.ascontiguousarray(
            xh[:, :, c * M:(c + 1) * M, :]).reshape(2, 128, M * P)
        mmap["query"] = np.ascontiguousarray(query[c * M:(c + 1) * M])
        in_maps.append(mmap)
    res = run_bass_kernel_spmd(nc, in_maps, core_ids=list(range(NCORES)))
    outs = []
    for c in range(NCORES):
        ye = np.asarray(res.results[c]["y"]).reshape(128, 2, M)
        outs.append(ye.transpose(2, 1, 0).reshape(M, C).astype(np.float32))
    return np.concatenate(outs, axis=0).reshape(B, N, C)


if __name__ == "__main__":
    rng = np.random.default_rng(0)
    ins = {
        "x": rng.standard_normal((B, N, 1, P, C), dtype=np.float32),
        "query": rng.standard_normal((B, N, C), dtype=np.float32),
        "ln_w": np.full((C,), C ** -0.5, np.float32),
        "ln_b": np.zeros((C,), np.float32),
        "w1": (rng.standard_normal((HID, C)) * 0.02).astype(np.float32),
        "b1": np.zeros((HID,), np.float32),
        "w2": (rng.standard_normal((TOTAL, HID)) * 0.02).astype(np.float32),
        "b2": (rng.standard_normal((TOTAL,)) * 0.05).astype(np.float32),
        "m_beta": np.zeros((C,), np.float32),
        "s_beta": np.zeros((OP,), np.float32),
        "proj_w": (rng.standard_normal((C, OP * C)) * 0.02).astype(np.float32),
        "proj_b": np.zeros((C,), np.float32),
    }
    out = kernel(**ins)
    print("ran", out.shape, out.dtype)
